# revision 14
# baseline (speedup 1.0000x reference)
"""CoAttention ImageDNS kernel for Trainium2 (8 NeuronCores, Bass/Tile).

Math: the reference computes two additive-attention blocks. In both, the
softmax'd score is  score[b, q, k] = f(q-side)[b, q] + g(k-side)[b, k] + c,
and softmax over k is invariant to the q-dependent (and constant) terms, so
the attention weights are independent of the query index:

  visual_att[b, s, :]  = softmax_r( wB . tanh(W_i1 @ img[b, r]) )
  textual_att[b, i, :] = softmax_j( wD . tanh(W_d2 @ dns[b, j]) )

Hence both outputs are per-batch rank-1 broadcasts:

  att_img_features[b, s, :] = visual_att[b]  @ img[b]   (same for all s)
  att_dns_features[b, i, :] = textual_att[b] @ dns[b]   (same for all i)

W_d1/b_d1/w_att1[:H]/b_att1/W_i2/b_i2/w_att2[:H]/b_att2 cancel entirely.

Sharding: pure data-parallel over batch, 4 batches per core, no collectives.

Device dataflow (per core), designed around the bf16 PE streaming roofline
(~216 ns per K=128 N=512 matmul; LDWEIGHTS hides under the stream):
  - Only the h-transposed activations xt[h, row] are loaded (bf16); the rows
    of all 4 batches are packed along the free dim so row-chunks of 128 have
    no per-batch padding waste (784 img rows -> 7 chunks, 2048 dns -> 16).
  - Projection: chunk-major MMs, activations stationary, weights streaming.
  - score chain per chunk: tanh (ScalarE, bf16 out) -> scalar_tensor_tensor
    with the wB/wD broadcast row + free-dim accumulate (VectorE) giving the
    score column [rk, 1]; a PE transpose turns it into a score row; exp on
    ScalarE writes the per-side exp row [1, rows].
  - per batch: row-sum + reciprocal + normalize (VectorE), partition-
    broadcast of the normalized attention row (GpSimd), then stage-2 as 8
    STT free-dim-accumulate ops over xt (VectorE) - no xn loads, no PE.
  - outputs: one [H] vector per (batch, side), PE-transposed to [8, 128]
    and DMA'd out (32 KB total instead of 16.8 MB of broadcast rows); the
    host broadcasts to the full (B, S, H) shape during unshard.
  - PE-queue ops that depend on the VectorE chain (the transposes) are
    emitted 1-2 chunks late so the in-order PE queue never stalls.
"""

import sys
import numpy as np
import ml_dtypes

_BF16 = ml_dtypes.bfloat16

for _p in ("/opt/trn_rl_repo", "/root/.axon_site/_ro/trn_rl_repo"):
    if _p not in sys.path:
        sys.path.append(_p)

B, S, R, H = 32, 512, 196, 1024
NCORES = 8
BLOC = B // NCORES          # batches per core
OC = 512                    # output-chunk (one fp32 PSUM bank)
HC = H // 128               # contraction chunks
DTOT = BLOC * S             # packed dns rows per core (2048)
ITOT = BLOC * R             # packed img rows per core (784)

_CACHE = {}


def _row_chunks(n):
    out, o = [], 0
    while o < n:
        out.append((o, min(128, n - o)))
        o += 128
    return out


def build_nc():
    from concourse import bacc, mybir
    from concourse import tile

    f32, f16 = mybir.dt.float32, mybir.dt.bfloat16
    Act = mybir.ActivationFunctionType
    Alu = mybir.AluOpType
    Ax = mybir.AxisListType

    nc = bacc.Bacc("TRN2", target_bir_lowering=False, debug=False)

    xt_dns_d = nc.dram_tensor("xt_dns", [HC, 128, DTOT], f16, kind="ExternalInput")
    xt_img_d = nc.dram_tensor("xt_img", [HC, 128, ITOT], f16, kind="ExternalInput")
    wt_i1_d = nc.dram_tensor("wt_i1", [HC, 128, H], f16, kind="ExternalInput")
    wt_d2_d = nc.dram_tensor("wt_d2", [HC, 128, H], f16, kind="ExternalInput")
    wr_b_d = nc.dram_tensor("wrow_b", [128, H], f16, kind="ExternalInput")
    wr_d_d = nc.dram_tensor("wrow_d", [128, H], f16, kind="ExternalInput")
    ident_d = nc.dram_tensor("ident", [128, 128], f32, kind="ExternalInput")
    out_d = nc.dram_tensor("out_all", [2, BLOC, HC, 128], f32, kind="ExternalOutput")

    IMG_RCS = _row_chunks(ITOT)      # 7 chunks (6x128 + 16)
    DNS_RCS = _row_chunks(DTOT)      # 16 chunks
    # batch -> last chunk holding its rows (chunk boundaries don't align with
    # batch boundaries on the img side)
    img_last_chunk = [((b + 1) * R - 1) // 128 for b in range(BLOC)]
    dns_last_chunk = [((b + 1) * S - 1) // 128 for b in range(BLOC)]

    with tile.TileContext(nc) as tc:
        with (
            tc.tile_pool(name="const", bufs=1) as cpool,
            tc.tile_pool(name="th", bufs=3) as thpool,
            tc.tile_pool(name="scr", bufs=2) as jpool,
            tc.tile_pool(name="small", bufs=3) as spool,
            tc.tile_pool(name="attp", bufs=4) as apool,
            tc.tile_pool(name="attacc", bufs=8) as accpool,
            tc.tile_pool(name="bc", bufs=2) as bpool,
            tc.tile_pool(name="pp", bufs=3, space="PSUM") as ppool,
            tc.tile_pool(name="tp", bufs=2, space="PSUM") as tpool,
        ):
            # ---- persistent SBUF tiles ----
            xt_img = cpool.tile([128, HC, ITOT], f16, name="xt_img_sb")
            xt_dns = cpool.tile([128, HC, DTOT], f16, name="xt_dns_sb")
            wt_img = cpool.tile([128, HC, H], f16, name="wt_img_sb")
            wt_dns = cpool.tile([128, HC, H], f16, name="wt_dns_sb")
            wrb = {"img": cpool.tile([128, H], f16, name="wrb_img"),
                   "dns": cpool.tile([128, H], f16, name="wrb_dns")}
            ident = cpool.tile([128, 128], f32, name="ident_sb")
            # exp rows in bf16: keeps every tensor operand of the stage-2
            # STTs 16-bit (2x DVE mode) at ~0.4% relative cost on the weights
            erow = {"img": cpool.tile([1, ITOT], f16, name="erow_img"),
                    "dns": cpool.tile([1, DTOT], f16, name="erow_dns")}

            SD = {
                "img": dict(xt=xt_img, wt=wt_img, rcs=IMG_RCS, rows=R,
                            last=img_last_chunk, oidx=0),
                "dns": dict(xt=xt_dns, wt=wt_dns, rcs=DNS_RCS, rows=S,
                            last=dns_last_chunk, oidx=1),
            }

            # ---- PE warmup: ~10 dummy matmuls on scratch data get the HAM
            # clock gate to 8/8 (2.4 GHz) during the first ~4us, which is
            # DMA-bound anyway; real matmuls then run warm from the start.
            warm_sb = cpool.tile([128, OC], f16, name="warm_sb")
            nc.vector.memset(warm_sb[:, :], 0.0)
            warm_ps = ppool.tile([128, H], f32, name="warm_ps", tag="pp")
            for i in range(10):
                nc.tensor.matmul(warm_ps[:, 0:OC], lhsT=warm_sb[:, 0:128],
                                 rhs=warm_sb[:, :], start=True, stop=True)

            # ---- input DMAs ----
            # three issue queues in parallel from t=0:
            #   sync:   img xt per-hc, then dns xt in two column halves
            #   scalar: ident + wB/wD rows + img wt (ScalarE compute starts
            #           only at ~10us, after these are issued)
            #   gpsimd: dns wt
            nc.scalar.dma_start(out=ident[:, :], in_=ident_d[:, :])
            nc.scalar.dma_start(out=wrb["img"][:, :], in_=wr_b_d[:, :])
            nc.scalar.dma_start(out=wrb["dns"][:, :], in_=wr_d_d[:, :])
            for hc in range(HC):
                nc.scalar.dma_start(out=wt_img[:, hc, :], in_=wt_i1_d[hc])
                nc.sync.dma_start(out=xt_img[:, hc, :], in_=xt_img_d[hc])
                nc.gpsimd.dma_start(out=wt_dns[:, hc, :], in_=wt_d2_d[hc])
            HD = DTOT // 2
            for half in range(2):
                cs = slice(half * HD, (half + 1) * HD)
                for hc in range(HC):
                    nc.sync.dma_start(out=xt_dns[:, hc, cs],
                                      in_=xt_dns_d[hc][:, cs])

            # ---- per-chunk pieces ----
            tcols = {}
            tps_tiles = {}

            def emit_mm(side, ci):
                """proj MMs for one chunk + its (non-PE) score chain."""
                sd = SD[side]
                r0, rk = sd["rcs"][ci]
                ps = ppool.tile([128, H], f32, name=f"ps_{side}_{ci}", tag="pp")
                for hc in range(HC):
                    lhs = sd["xt"][:, hc, r0:r0 + rk]
                    for oc in range(2):
                        nc.tensor.matmul(
                            ps[0:rk, oc * OC:(oc + 1) * OC],
                            lhsT=lhs,
                            rhs=sd["wt"][:, hc, oc * OC:(oc + 1) * OC],
                            start=(hc == 0), stop=(hc == HC - 1))
                emit_chain(side, ci, ps)

            def emit_mm_prologue(side, cis):
                """hc-major MMs over several chunks: consumes the per-hc input
                DMAs progressively so the PE starts ~1.5us into the kernel."""
                sd = SD[side]
                pss = {}
                for ci in cis:
                    pss[ci] = ppool.tile([128, H], f32, name=f"ps_{side}_{ci}",
                                         tag="pp")
                for hc in range(HC):
                    for ci in cis:
                        r0, rk = sd["rcs"][ci]
                        lhs = sd["xt"][:, hc, r0:r0 + rk]
                        for oc in range(2):
                            nc.tensor.matmul(
                                pss[ci][0:rk, oc * OC:(oc + 1) * OC],
                                lhsT=lhs,
                                rhs=sd["wt"][:, hc, oc * OC:(oc + 1) * OC],
                                start=(hc == 0), stop=(hc == HC - 1))
                for ci in cis:
                    emit_chain(side, ci, pss[ci])

            def emit_chain(side, ci, ps):
                """tanh -> weighted free-dim reduce -> score column [rk, 1]."""
                sd = SD[side]
                r0, rk = sd["rcs"][ci]
                th = thpool.tile([128, H], f16, name=f"th_{side}_{ci}", tag="th")
                nc.scalar.activation(th[0:rk, :], ps[0:rk, :], Act.Tanh)
                scr = jpool.tile([128, H], f16, name=f"scr_{side}_{ci}", tag="scr")
                tcol = spool.tile([128, 1], f32, name=f"tc_{side}_{ci}", tag="tcol")
                nc.vector.scalar_tensor_tensor(
                    out=scr[0:rk, :], in0=th[0:rk, :], scalar=1.0,
                    in1=wrb[side][0:rk, :], op0=Alu.mult, op1=Alu.mult,
                    accum_out=tcol[0:rk, :])
                tcols[(side, ci)] = tcol

            def emit_T(side, ci):
                """PE transpose of the score column -> exp row slice.
                Emitted >=1 chunk after emit_mm so the PE queue never waits
                on the VectorE chain."""
                sd = SD[side]
                r0, rk = sd["rcs"][ci]
                tcol = tcols[(side, ci)]
                tps = tpool.tile([8, 128], f32, name=f"tps_{side}_{ci}", tag="tp")
                nc.tensor.transpose(tps[0:1, 0:rk], tcol[0:rk, 0:1],
                                    ident[0:rk, 0:rk])
                nc.scalar.activation(erow[side][0:1, r0:r0 + rk],
                                     tps[0:1, 0:rk], Act.Exp)

            # ---- stage 2, split into head/tail parts ----
            # att[h] = (sum_r exp_r x[h,r]) / sum_r exp_r.  The unnormalized
            # partials only need the exp row, so the head part (all chunks of
            # the batch but the last) runs a chunk earlier than a normalized
            # formulation would allow; only the last chunk's sliver plus the
            # finalize remains on the critical tail.
            attps, atts = {}, {}

            def split_batch(side, b):
                # only the last-finishing batch of a side benefits from the
                # head/tail split; single-part elsewhere keeps the DVE
                # instruction count (and its fixed overheads) low
                return b == BLOC - 1

            def emit_part(side, b, head):
                sd = SD[side]
                rows = sd["rows"]
                b0, bend = b * rows, (b + 1) * rows
                if split_batch(side, b):
                    split = max(b0, sd["last"][b] * 128)
                    lo, hi = (b0, split) if head else (split, bend)
                else:
                    if head:
                        return
                    lo, hi = b0, bend
                if hi <= lo:
                    return
                w = hi - lo
                key = (side, b)
                if key not in attps:
                    attp = accpool.tile([128, HC * 2], f32,
                                        name=f"attp_{side}_{b}", tag="attp")
                    nc.vector.memset(attp[:, :], 0.0)
                    attps[key] = attp
                attp = attps[key]
                pi = 0 if head else 1
                abc = bpool.tile([128, w], f16, name=f"abc_{side}_{b}_{pi}",
                                 tag=f"abc_{int(head)}_{side}")
                nc.gpsimd.partition_broadcast(abc[:, :], erow[side][0:1, lo:hi])
                for hc in range(HC):
                    sj = jpool.tile([128, w], f16, name=f"sj_{side}_{b}_{hc}_{pi}",
                                    tag=f"sj_{side}")
                    nc.vector.scalar_tensor_tensor(
                        out=sj[:, :], in0=sd["xt"][:, hc, lo:hi],
                        scalar=1.0, in1=abc[:, :], op0=Alu.mult, op1=Alu.mult,
                        accum_out=attp[:, hc * 2 + pi:hc * 2 + pi + 1])

            def emit_finalize(side, b):
                sd = SD[side]
                rows = sd["rows"]
                b0 = b * rows
                ssum = spool.tile([1, 1], f32, name=f"ss_{side}_{b}", tag="ssum")
                nc.vector.tensor_reduce(out=ssum[0:1, 0:1],
                                        in_=erow[side][0:1, b0:b0 + rows],
                                        axis=Ax.X, op=Alu.add)
                rcp = spool.tile([1, 1], f32, name=f"rc_{side}_{b}", tag="rcp")
                nc.vector.reciprocal(rcp[0:1, 0:1], ssum[0:1, 0:1])
                rb = spool.tile([128, 1], f32, name=f"rb_{side}_{b}", tag="rb")
                nc.gpsimd.partition_broadcast(rb[:, 0:1], rcp[0:1, 0:1])
                attp = attps[(side, b)]
                attf = apool.tile([128, HC], f32, name=f"attf_{side}_{b}",
                                  tag="attf")
                nc.vector.tensor_reduce(
                    out=attf[:, :],
                    in_=attp[:, :].rearrange("p (h t) -> p h t", t=2),
                    axis=Ax.X, op=Alu.add)
                attsc = apool.tile([128, HC], f32, name=f"attsc_{side}_{b}",
                                   tag="attsc")
                nc.scalar.activation(attsc[:, :], attf[:, :], Act.Copy,
                                     scale=rb[:, 0:1])
                atts[(side, b)] = attsc

            def emit_attT(side, b):
                """PE transpose of the output column-tile + writeback."""
                att = atts[(side, b)]
                atp = tpool.tile([8, 128], f32, name=f"atp_{side}_{b}", tag="tp")
                nc.tensor.transpose(atp[0:8, 0:128], att[:, 0:HC],
                                    ident[:, :])
                osb = spool.tile([8, 128], f32, name=f"osb_{side}_{b}", tag="osb")
                nc.scalar.activation(osb[:, :], atp[:, :], Act.Copy)
                nc.sync.dma_start(out=out_d[SD[side]["oidx"], b], in_=osb[:, :])

            def after_T(side, ci):
                for b in range(BLOC):
                    if SD[side]["last"][b] - 1 == ci:
                        emit_part(side, b, head=True)
                    if SD[side]["last"][b] == ci:
                        emit_part(side, b, head=False)
                        emit_finalize(side, b)

            # ---- emission schedule ----
            # PE order: img prologue (c0-2, hc-major, DMA-paced), img c3-c5,
            # all dns chunks (their xt arrives while img computes), and the
            # 16-row img c6 last so the end-of-kernel chain is short.  Score
            # transposes are deferred >=1 chunk; output transposes >=2.
            emit_mm_prologue("img", [0, 1, 2])
            emit_mm("img", 3)
            for ci in (0, 1, 2):
                emit_T("img", ci); after_T("img", ci)
            emit_mm("img", 4); emit_T("img", 3); after_T("img", 3)
            emit_mm("img", 5); emit_T("img", 4); after_T("img", 4)
            emit_mm("dns", 0); emit_T("img", 5); after_T("img", 5)
            emit_mm("dns", 1); emit_attT("img", 0)
            emit_mm("dns", 2); emit_attT("img", 1); emit_T("dns", 0); after_T("dns", 0)
            emit_mm("dns", 3); emit_attT("img", 2); emit_T("dns", 1); after_T("dns", 1)
            attT_slot = {7: ("dns", 0), 11: ("dns", 1), 15: ("dns", 2)}
            for ci in range(4, 16):
                emit_mm("dns", ci)
                if ci in attT_slot:
                    emit_attT(*attT_slot[ci])
                emit_T("dns", ci - 2); after_T("dns", ci - 2)
            emit_mm("img", 6)
            emit_T("dns", 14); after_T("dns", 14)
            emit_T("dns", 15); after_T("dns", 15)
            emit_T("img", 6); after_T("img", 6)
            emit_attT("dns", 3)
            emit_attT("img", 3)

    nc.compile()
    return nc


def _get_nc():
    if "nc" not in _CACHE:
        _CACHE["nc"] = build_nc()
    return _CACHE["nc"]


def make_in_maps(inputs):
    dns = np.asarray(inputs["dns_feature"], dtype=np.float32)
    img = np.asarray(inputs["img_features"], dtype=np.float32)
    W_i1 = np.asarray(inputs["W_i1"], dtype=np.float32)
    W_d2 = np.asarray(inputs["W_d2"], dtype=np.float32)
    wB = np.asarray(inputs["w_att1"], dtype=np.float32)[H:]
    wD = np.asarray(inputs["w_att2"], dtype=np.float32)[H:]

    wt_i1 = np.ascontiguousarray(W_i1.T).reshape(HC, 128, H).astype(_BF16)
    wt_d2 = np.ascontiguousarray(W_d2.T).reshape(HC, 128, H).astype(_BF16)
    wr_b = np.ascontiguousarray(np.broadcast_to(wB, (128, H)).astype(_BF16))
    wr_d = np.ascontiguousarray(np.broadcast_to(wD, (128, H)).astype(_BF16))
    ident = np.eye(128, dtype=np.float32)

    in_maps = []
    for k in range(NCORES):
        sl = slice(k * BLOC, (k + 1) * BLOC)
        xd = np.ascontiguousarray(
            dns[sl].transpose(2, 0, 1).reshape(HC, 128, DTOT).astype(_BF16))
        xi = np.ascontiguousarray(
            img[sl].transpose(2, 0, 1).reshape(HC, 128, ITOT).astype(_BF16))
        in_maps.append({
            "xt_dns": xd, "xt_img": xi,
            "wt_i1": wt_i1, "wt_d2": wt_d2,
            "wrow_b": wr_b, "wrow_d": wr_d, "ident": ident,
        })
    return in_maps


def kernel(**inputs):
    from concourse.bass_utils import run_bass_kernel_spmd

    nc = _get_nc()
    in_maps = make_in_maps(inputs)
    res = run_bass_kernel_spmd(nc, in_maps, list(range(NCORES))).results
    out = np.stack([np.asarray(res[k]["out_all"]) for k in range(NCORES)])
    img_rows = out[:, 0].reshape(B, H)
    dns_rows = out[:, 1].reshape(B, H)
    att_dns = np.ascontiguousarray(
        np.broadcast_to(dns_rows[:, None, :], (B, S, H)))
    att_img = np.ascontiguousarray(
        np.broadcast_to(img_rows[:, None, :], (B, S, H)))
    return att_dns, att_img


# revision 15
# speedup vs baseline: 1.0963x; 1.0963x over previous
"""CoAttention ImageDNS kernel for Trainium2 (8 NeuronCores, Bass/Tile).

Math: the reference computes two additive-attention blocks. In both, the
softmax'd score is  score[b, q, k] = f(q-side)[b, q] + g(k-side)[b, k] + c,
and softmax over k is invariant to the q-dependent (and constant) terms, so
the attention weights are independent of the query index:

  visual_att[b, s, :]  = softmax_r( wB . tanh(W_i1 @ img[b, r]) )
  textual_att[b, i, :] = softmax_j( wD . tanh(W_d2 @ dns[b, j]) )

Hence both outputs are per-batch rank-1 broadcasts:

  att_img_features[b, s, :] = visual_att[b]  @ img[b]   (same for all s)
  att_dns_features[b, i, :] = textual_att[b] @ dns[b]   (same for all i)

W_d1/b_d1/w_att1[:H]/b_att1/W_i2/b_i2/w_att2[:H]/b_att2 cancel entirely.

Sharding: pure data-parallel over batch, 4 batches per core, no collectives.
Matmul operands are bf16 (fp16 streams at half rate on the trn2 PE; bf16
end-to-end rel err ~3e-3 vs the fp32 reference); accumulation is fp32 in
PSUM, softmax/normalization in fp32.
"""

import sys
import numpy as np
import ml_dtypes

_BF16 = ml_dtypes.bfloat16

for _p in ("/opt/trn_rl_repo", "/root/.axon_site/_ro/trn_rl_repo"):
    if _p not in sys.path:
        sys.path.append(_p)

B, S, R, H = 32, 512, 196, 1024
NCORES = 8
BLOC = B // NCORES          # batches per core
OC = 512                    # output-chunk (one fp32 PSUM bank)
HC = H // 128               # contraction chunks

_CACHE = {}


def _row_chunks(n):
    out, o = [], 0
    while o < n:
        out.append((o, min(128, n - o)))
        o += 128
    return out


def build_nc():
    from concourse import bacc, mybir
    from concourse import tile

    f32, f16 = mybir.dt.float32, mybir.dt.bfloat16
    Act = mybir.ActivationFunctionType
    Alu = mybir.AluOpType

    nc = bacc.Bacc("TRN2", target_bir_lowering=False, debug=False)

    RP = 256  # img row count padded to a partition multiple for single-DMA loads
    xt_dns = nc.dram_tensor("xt_dns", [BLOC, HC, 128, S], f16, kind="ExternalInput")
    xn_dns = nc.dram_tensor("xn_dns", [BLOC, S, H], f16, kind="ExternalInput")
    xt_img = nc.dram_tensor("xt_img", [BLOC, HC, 128, R], f16, kind="ExternalInput")
    xn_img = nc.dram_tensor("xn_img", [BLOC, RP, H], f16, kind="ExternalInput")
    wt_i1 = nc.dram_tensor("wt_i1", [HC, 128, H], f16, kind="ExternalInput")
    wt_d2 = nc.dram_tensor("wt_d2", [HC, 128, H], f16, kind="ExternalInput")
    wrow_b = nc.dram_tensor("wrow_b", [128, H], f32, kind="ExternalInput")
    wrow_d = nc.dram_tensor("wrow_d", [128, H], f32, kind="ExternalInput")
    out_dns = nc.dram_tensor("out_dns", [BLOC, S, H], f32, kind="ExternalOutput")
    out_img = nc.dram_tensor("out_img", [BLOC, S, H], f32, kind="ExternalOutput")

    with tile.TileContext(nc) as tc:
        with (
            tc.tile_pool(name="const", bufs=1) as cpool,
            tc.tile_pool(name="xts", bufs=2) as xtpool,
            tc.tile_pool(name="xns", bufs=2) as xnpool,
            tc.tile_pool(name="work", bufs=3) as wpool,
            tc.tile_pool(name="small", bufs=12) as spool,
            tc.tile_pool(name="outs", bufs=2) as opool,
            tc.tile_pool(name="pp", bufs=3, space="PSUM") as ppool,
            tc.tile_pool(name="ps", bufs=2, space="PSUM") as pstat,
        ):
            # lazy const loads: weight DMAs are interleaved with the first
            # activation loads (per-hc) at first use, so the first projection
            # group's dependencies land early in the queue
            wt_sb, wrow_sb = {}, {}

            def get_wt(nm):
                if nm not in wt_sb:
                    w = cpool.tile([128, HC * H], f16, name=f"wt_{nm}_sb")
                    wt_sb[nm] = w
                return wt_sb[nm]

            def load_wt_chunk(nm, hc):
                dram = {"i1": wt_i1, "d2": wt_d2}[nm]
                w = wt_sb[nm]
                nc.sync.dma_start(out=w[:, hc * H:(hc + 1) * H], in_=dram[hc])

            def get_wrow(nm):
                if nm not in wrow_sb:
                    dram = {"b": wrow_b, "d": wrow_d}[nm]
                    w = cpool.tile([128, H], f32, name=f"wrow_{nm}_sb")
                    nc.sync.dma_start(out=w[:, :], in_=dram[:, :])
                    wrow_sb[nm] = w
                return wrow_sb[nm]

            ones_col = cpool.tile([128, 1], f16, name="ones_col")
            nc.vector.memset(ones_col[:, :], 1.0)
            ones_row = cpool.tile([1, 128], f32, name="ones_row")
            nc.vector.memset(ones_row[:, :], 1.0)

            for b in range(BLOC):
                for side in ("img", "dns"):
                    n_rows = R if side == "img" else S
                    xt_d = xt_img if side == "img" else xt_dns
                    xn_d = xn_img if side == "img" else xn_dns
                    wt_name = "i1" if side == "img" else "d2"
                    load_wt = wt_name not in wt_sb
                    wt = get_wt(wt_name)
                    out_d = out_img if side == "img" else out_dns
                    rcs = _row_chunks(n_rows)

                    # -- loads: on a weight's first use, interleave per-hc wt/xt
                    # chunks so the first projection group's deps land first;
                    # afterwards one 3D DMA covers the whole xt tile --
                    xt_t = xtpool.tile([128, HC * n_rows], f16,
                                       name=f"xt_{side}_{b}", tag=f"xt_{side}")
                    if load_wt:
                        for hc in range(HC):
                            load_wt_chunk(wt_name, hc)
                            nc.sync.dma_start(
                                out=xt_t[:, hc * n_rows:(hc + 1) * n_rows],
                                in_=xt_d[b, hc])
                    else:
                        nc.sync.dma_start(
                            out=xt_t.rearrange("p (hc m) -> p hc m", hc=HC),
                            in_=xt_d[b].rearrange("hc p m -> p hc m"))

                    # -- projection, tanh, weighted o-reduction, exp --
                    # xn / wrow loads are issued after the first proj group so
                    # the projection's own dependencies lead the DMA queues
                    acols = []
                    xn_ts = []
                    wr = None
                    s_ps = pstat.tile([1, 1], f32, name=f"s_{side}_{b}", tag="stat")
                    for ci, (r0, rk) in enumerate(rcs):
                        ps = ppool.tile([128, H], f32, name=f"proj_{side}_{ci}_{b}",
                                        tag="pp")
                        for hc in range(HC):
                            lhs = xt_t[:, hc * n_rows + r0: hc * n_rows + r0 + rk]
                            for oc in range(2):
                                nc.tensor.matmul(
                                    ps[0:rk, oc * OC:(oc + 1) * OC],
                                    lhsT=lhs,
                                    rhs=wt[:, hc * H + oc * OC: hc * H + (oc + 1) * OC],
                                    start=(hc == 0), stop=(hc == HC - 1))
                        if ci == 0:
                            nrc = len(rcs)
                            xn_t = xnpool.tile([128, nrc * H], f16,
                                               name=f"xn_{side}_{b}", tag=f"xn_{side}")
                            nc.sync.dma_start(
                                out=xn_t.rearrange("p (rc n) -> p rc n", rc=nrc),
                                in_=xn_d[b, 0:nrc * 128, :]
                                .rearrange("(rc p) n -> p rc n", p=128))
                            xn_ts = [xn_t[:, cj * H:(cj + 1) * H] for cj in range(nrc)]
                            wr = get_wrow("b" if side == "img" else "d")
                        th = wpool.tile([128, H], f32, name=f"th_{side}_{ci}_{b}", tag="th")
                        nc.scalar.activation(th[0:rk, :], ps[0:rk, :], Act.Tanh)
                        scr = wpool.tile([128, H], f32, name=f"scr_{side}_{ci}_{b}",
                                         tag="scr", bufs=2)
                        tcol = spool.tile([128, 1], f32, name=f"tc_{side}_{ci}_{b}", tag="tcol")
                        nc.vector.scalar_tensor_tensor(
                            out=scr[0:rk, :], in0=th[0:rk, :], scalar=1.0,
                            in1=wr[0:rk, :], op0=Alu.mult, op1=Alu.mult,
                            accum_out=tcol[0:rk, :])
                        acol = spool.tile([128, 1], f16, name=f"a_{side}_{ci}_{b}",
                                          tag=f"acol_{side}_{ci}")
                        nc.scalar.activation(acol[0:rk, :], tcol[0:rk, :], Act.Exp)
                        acols.append((acol, rk))
                        nc.tensor.matmul(
                            s_ps[0:1, 0:1], lhsT=acol[0:rk, 0:1], rhs=ones_col[0:rk, 0:1],
                            start=(ci == 0), stop=(ci == len(rcs) - 1))

                    # -- 1/sum, broadcast to 128 partitions (idle GPSIMD) --
                    r_sb = spool.tile([1, 1], f32, name=f"r_{side}_{b}", tag="r")
                    nc.vector.reciprocal(r_sb[0:1, 0:1], s_ps[0:1, 0:1])
                    rb_sb = spool.tile([128, 1], f32, name=f"rbs_{side}_{b}", tag="rb")
                    nc.gpsimd.partition_broadcast(rb_sb[:, 0:1], r_sb[0:1, 0:1])

                    # -- stage 2: out[s, h] = sum_r a_r x[r, h], all 128 s at once --
                    att_ps = ppool.tile([128, H], f32, name=f"att_{side}_{b}", tag="pp")
                    for h2 in range(2):
                        for ci, (r0, rk) in enumerate(rcs):
                            acol, _ = acols[ci]
                            nc.tensor.matmul(
                                att_ps[:, h2 * OC:(h2 + 1) * OC],
                                lhsT=acol[0:rk, 0:1].to_broadcast((rk, 128)),
                                rhs=xn_ts[ci][0:rk, h2 * OC:(h2 + 1) * OC],
                                start=(ci == 0), stop=(ci == len(rcs) - 1))
                    att_sb = opool.tile([128, H], f32, name=f"attsb_{side}_{b}",
                                        tag=f"att_{side}")
                    for h2 in range(2):
                        nc.scalar.activation(att_sb[:, h2 * OC:(h2 + 1) * OC],
                                             att_ps[:, h2 * OC:(h2 + 1) * OC],
                                             Act.Copy, scale=rb_sb[:, 0:1])
                        # broadcast DMA: all 512 output rows of this h-half
                        nc.sync.dma_start(
                            out=out_d[b, :, h2 * OC:(h2 + 1) * OC]
                            .rearrange("(sc p) n -> p sc n", p=128),
                            in_=att_sb[:, h2 * OC:(h2 + 1) * OC]
                            .rearrange("p (o n) -> p o n", o=1)
                            .to_broadcast((128, S // 128, OC)))
    nc.compile()
    return nc


def _get_nc():
    if "nc" not in _CACHE:
        _CACHE["nc"] = build_nc()
    return _CACHE["nc"]


def make_in_maps(inputs):
    dns = np.ascontiguousarray(np.asarray(inputs["dns_feature"], dtype=np.float32))
    img = np.ascontiguousarray(np.asarray(inputs["img_features"], dtype=np.float32))
    W_i1 = np.asarray(inputs["W_i1"], dtype=np.float32)
    W_d2 = np.asarray(inputs["W_d2"], dtype=np.float32)
    wB = np.asarray(inputs["w_att1"], dtype=np.float32)[H:]
    wD = np.asarray(inputs["w_att2"], dtype=np.float32)[H:]

    wt_i1 = np.ascontiguousarray(W_i1.T).reshape(HC, 128, H).astype(_BF16)
    wt_d2 = np.ascontiguousarray(W_d2.T).reshape(HC, 128, H).astype(_BF16)
    wrow_b = np.ascontiguousarray(np.broadcast_to(wB, (128, H)))
    wrow_d = np.ascontiguousarray(np.broadcast_to(wD, (128, H)))

    xt_dns = np.ascontiguousarray(
        dns.transpose(0, 2, 1).reshape(B, HC, 128, S).astype(_BF16))
    xt_img = np.ascontiguousarray(
        img.transpose(0, 2, 1).reshape(B, HC, 128, R).astype(_BF16))
    xn_dns = dns.astype(_BF16)
    xn_img = np.zeros((B, 256, H), dtype=_BF16)
    xn_img[:, :R, :] = img.astype(_BF16)

    in_maps = []
    for k in range(NCORES):
        sl = slice(k * BLOC, (k + 1) * BLOC)
        in_maps.append({
            "xt_dns": np.ascontiguousarray(xt_dns[sl]),
            "xn_dns": np.ascontiguousarray(xn_dns[sl]),
            "xt_img": np.ascontiguousarray(xt_img[sl]),
            "xn_img": np.ascontiguousarray(xn_img[sl]),
            "wt_i1": wt_i1,
            "wt_d2": wt_d2,
            "wrow_b": wrow_b,
            "wrow_d": wrow_d,
        })
    return in_maps


def kernel(**inputs):
    from concourse.bass_utils import run_bass_kernel_spmd

    nc = _get_nc()
    in_maps = make_in_maps(inputs)
    res = run_bass_kernel_spmd(nc, in_maps, list(range(NCORES))).results
    att_dns = np.concatenate([res[k]["out_dns"] for k in range(NCORES)], axis=0)
    att_img = np.concatenate([res[k]["out_img"] for k in range(NCORES)], axis=0)
    return att_dns, att_img



# revision 16
# speedup vs baseline: 1.1687x; 1.0661x over previous
"""CoAttention ImageDNS kernel for Trainium2 (8 NeuronCores, Bass/Tile).

Math: the reference computes two additive-attention blocks. In both, the
softmax'd score is  score[b, q, k] = f(q-side)[b, q] + g(k-side)[b, k] + c,
and softmax over k is invariant to the q-dependent (and constant) terms, so
the attention weights are independent of the query index:

  visual_att[b, s, :]  = softmax_r( wB . tanh(W_i1 @ img[b, r]) )
  textual_att[b, i, :] = softmax_j( wD . tanh(W_d2 @ dns[b, j]) )

Hence both outputs are per-batch rank-1 broadcasts:

  att_img_features[b, s, :] = visual_att[b]  @ img[b]   (same for all s)
  att_dns_features[b, i, :] = textual_att[b] @ dns[b]   (same for all i)

W_d1/b_d1/w_att1[:H]/b_att1/W_i2/b_i2/w_att2[:H]/b_att2 cancel entirely.

Sharding: pure data-parallel over batch, 4 batches per core, no collectives.

Device dataflow (per core), designed around the bf16 PE streaming roofline
(~216 ns per K=128 N=512 matmul; LDWEIGHTS hides under the stream):
  - Only the h-transposed activations xt[h, row] are loaded (bf16); the rows
    of all 4 batches are packed along the free dim so row-chunks of 128 have
    no per-batch padding waste (784 img rows -> 7 chunks, 2048 dns -> 16).
  - Projection: chunk-major MMs, activations stationary, weights streaming.
  - score chain per chunk: tanh (ScalarE, bf16 out) -> scalar_tensor_tensor
    with the wB/wD broadcast row + free-dim accumulate (VectorE) giving the
    score column [rk, 1]; a PE transpose turns it into a score row; exp on
    ScalarE writes the per-side exp row [1, rows].
  - per batch: row-sum + reciprocal + normalize (VectorE), partition-
    broadcast of the normalized attention row (GpSimd), then stage-2 as 8
    STT free-dim-accumulate ops over xt (VectorE) - no xn loads, no PE.
  - outputs: one [H] vector per (batch, side), PE-transposed to [8, 128]
    and DMA'd out (32 KB total instead of 16.8 MB of broadcast rows); the
    host broadcasts to the full (B, S, H) shape during unshard.
  - PE-queue ops that depend on the VectorE chain (the transposes) are
    emitted 1-2 chunks late so the in-order PE queue never stalls.
"""

import sys
import numpy as np
import ml_dtypes

_BF16 = ml_dtypes.bfloat16

for _p in ("/opt/trn_rl_repo", "/root/.axon_site/_ro/trn_rl_repo"):
    if _p not in sys.path:
        sys.path.append(_p)

B, S, R, H = 32, 512, 196, 1024
NCORES = 8
BLOC = B // NCORES          # batches per core
OC = 512                    # output-chunk (one fp32 PSUM bank)
HC = H // 128               # contraction chunks
DTOT = BLOC * S             # packed dns rows per core (2048)
ITOT = BLOC * R             # packed img rows per core (784)

_CACHE = {}


def _row_chunks(n):
    out, o = [], 0
    while o < n:
        out.append((o, min(128, n - o)))
        o += 128
    return out


def build_nc():
    from concourse import bacc, mybir
    from concourse import tile

    f32, f16 = mybir.dt.float32, mybir.dt.bfloat16
    Act = mybir.ActivationFunctionType
    Alu = mybir.AluOpType
    Ax = mybir.AxisListType

    nc = bacc.Bacc("TRN2", target_bir_lowering=False, debug=False)

    xt_dns_d = nc.dram_tensor("xt_dns", [HC, 128, DTOT], f16, kind="ExternalInput")
    xt_img_d = nc.dram_tensor("xt_img", [HC, 128, ITOT], f16, kind="ExternalInput")
    wt_i1_d = nc.dram_tensor("wt_i1", [HC, 128, H], f16, kind="ExternalInput")
    wt_d2_d = nc.dram_tensor("wt_d2", [HC, 128, H], f16, kind="ExternalInput")
    wr_b_d = nc.dram_tensor("wrow_b", [128, H], f16, kind="ExternalInput")
    wr_d_d = nc.dram_tensor("wrow_d", [128, H], f16, kind="ExternalInput")
    ident_d = nc.dram_tensor("ident", [128, 128], f32, kind="ExternalInput")
    out_d = nc.dram_tensor("out_all", [2, BLOC, HC, 128], f32, kind="ExternalOutput")

    IMG_RCS = _row_chunks(ITOT)      # 7 chunks (6x128 + 16)
    DNS_RCS = _row_chunks(DTOT)      # 16 chunks
    # batch -> last chunk holding its rows (chunk boundaries don't align with
    # batch boundaries on the img side)
    img_last_chunk = [((b + 1) * R - 1) // 128 for b in range(BLOC)]
    dns_last_chunk = [((b + 1) * S - 1) // 128 for b in range(BLOC)]

    with tile.TileContext(nc) as tc:
        with (
            tc.tile_pool(name="const", bufs=1) as cpool,
            tc.tile_pool(name="th", bufs=3) as thpool,
            tc.tile_pool(name="scr", bufs=2) as jpool,
            tc.tile_pool(name="small", bufs=3) as spool,
            tc.tile_pool(name="attp", bufs=4) as apool,
            tc.tile_pool(name="attacc", bufs=8) as accpool,
            tc.tile_pool(name="bc", bufs=2) as bpool,
            tc.tile_pool(name="pp", bufs=3, space="PSUM") as ppool,
            tc.tile_pool(name="tp", bufs=2, space="PSUM") as tpool,
        ):
            # ---- persistent SBUF tiles ----
            xt_img = cpool.tile([128, HC, ITOT], f16, name="xt_img_sb")
            xt_dns = cpool.tile([128, HC, DTOT], f16, name="xt_dns_sb")
            wt_img = cpool.tile([128, HC, H], f16, name="wt_img_sb")
            wt_dns = cpool.tile([128, HC, H], f16, name="wt_dns_sb")
            wrb = {"img": cpool.tile([128, H], f16, name="wrb_img"),
                   "dns": cpool.tile([128, H], f16, name="wrb_dns")}
            ident = cpool.tile([128, 128], f32, name="ident_sb")
            # exp rows in bf16: keeps every tensor operand of the stage-2
            # STTs 16-bit (2x DVE mode) at ~0.4% relative cost on the weights
            erow = {"img": cpool.tile([1, ITOT], f16, name="erow_img"),
                    "dns": cpool.tile([1, DTOT], f16, name="erow_dns")}

            SD = {
                "img": dict(xt=xt_img, wt=wt_img, rcs=IMG_RCS, rows=R,
                            last=img_last_chunk, oidx=0),
                "dns": dict(xt=xt_dns, wt=wt_dns, rcs=DNS_RCS, rows=S,
                            last=dns_last_chunk, oidx=1),
            }

            # ---- PE warmup: ~10 dummy matmuls on scratch data get the HAM
            # clock gate to 8/8 (2.4 GHz) during the first ~4us, which is
            # DMA-bound anyway; real matmuls then run warm from the start.
            warm_sb = cpool.tile([128, OC], f16, name="warm_sb")
            nc.vector.memset(warm_sb[:, :], 0.0)
            warm_ps = ppool.tile([128, H], f32, name="warm_ps", tag="pp")
            for i in range(10):
                nc.tensor.matmul(warm_ps[:, 0:OC], lhsT=warm_sb[:, 0:128],
                                 rhs=warm_sb[:, :], start=True, stop=True)

            # ---- input DMAs ----
            # three issue queues in parallel from t=0:
            #   sync:   img xt per-hc, then dns xt in two column halves
            #   scalar: ident + wB/wD rows + img wt (ScalarE compute starts
            #           only at ~10us, after these are issued)
            #   gpsimd: dns wt
            nc.scalar.dma_start(out=ident[:, :], in_=ident_d[:, :])
            nc.scalar.dma_start(out=wrb["img"][:, :], in_=wr_b_d[:, :])
            nc.scalar.dma_start(out=wrb["dns"][:, :], in_=wr_d_d[:, :])
            for hc in range(HC):
                nc.scalar.dma_start(out=wt_img[:, hc, :], in_=wt_i1_d[hc])
                nc.sync.dma_start(out=xt_img[:, hc, :], in_=xt_img_d[hc])
                nc.gpsimd.dma_start(out=wt_dns[:, hc, :], in_=wt_d2_d[hc])
            HD = DTOT // 2
            for half in range(2):
                cs = slice(half * HD, (half + 1) * HD)
                for hc in range(HC):
                    nc.sync.dma_start(out=xt_dns[:, hc, cs],
                                      in_=xt_dns_d[hc][:, cs])

            # ---- per-chunk pieces ----
            tcols = {}
            tps_tiles = {}

            def emit_mm(side, ci):
                """proj MMs for one chunk + its (non-PE) score chain."""
                sd = SD[side]
                r0, rk = sd["rcs"][ci]
                ps = ppool.tile([128, H], f32, name=f"ps_{side}_{ci}", tag="pp")
                for hc in range(HC):
                    lhs = sd["xt"][:, hc, r0:r0 + rk]
                    for oc in range(2):
                        nc.tensor.matmul(
                            ps[0:rk, oc * OC:(oc + 1) * OC],
                            lhsT=lhs,
                            rhs=sd["wt"][:, hc, oc * OC:(oc + 1) * OC],
                            start=(hc == 0), stop=(hc == HC - 1))
                emit_chain(side, ci, ps)

            def emit_mm_prologue(side, cis):
                """hc-major MMs over several chunks: consumes the per-hc input
                DMAs progressively so the PE starts ~1.5us into the kernel."""
                sd = SD[side]
                pss = {}
                for ci in cis:
                    pss[ci] = ppool.tile([128, H], f32, name=f"ps_{side}_{ci}",
                                         tag="pp")
                for hc in range(HC):
                    for ci in cis:
                        r0, rk = sd["rcs"][ci]
                        lhs = sd["xt"][:, hc, r0:r0 + rk]
                        for oc in range(2):
                            nc.tensor.matmul(
                                pss[ci][0:rk, oc * OC:(oc + 1) * OC],
                                lhsT=lhs,
                                rhs=sd["wt"][:, hc, oc * OC:(oc + 1) * OC],
                                start=(hc == 0), stop=(hc == HC - 1))
                for ci in cis:
                    emit_chain(side, ci, pss[ci])

            def emit_chain(side, ci, ps):
                """tanh -> weighted free-dim reduce -> score column [rk, 1]."""
                sd = SD[side]
                r0, rk = sd["rcs"][ci]
                th = thpool.tile([128, H], f16, name=f"th_{side}_{ci}", tag="th")
                nc.scalar.activation(th[0:rk, :], ps[0:rk, :], Act.Tanh)
                scr = jpool.tile([128, H], f16, name=f"scr_{side}_{ci}", tag="scr")
                tcol = spool.tile([128, 1], f32, name=f"tc_{side}_{ci}", tag="tcol")
                nc.vector.scalar_tensor_tensor(
                    out=scr[0:rk, :], in0=th[0:rk, :], scalar=1.0,
                    in1=wrb[side][0:rk, :], op0=Alu.mult, op1=Alu.mult,
                    accum_out=tcol[0:rk, :])
                tcols[(side, ci)] = tcol

            def emit_T(side, ci):
                """PE transpose of the score column -> exp row slice.
                Emitted >=1 chunk after emit_mm so the PE queue never waits
                on the VectorE chain."""
                sd = SD[side]
                r0, rk = sd["rcs"][ci]
                tcol = tcols[(side, ci)]
                tps = tpool.tile([8, 128], f32, name=f"tps_{side}_{ci}", tag="tp")
                nc.tensor.transpose(tps[0:1, 0:rk], tcol[0:rk, 0:1],
                                    ident[0:rk, 0:rk])
                nc.scalar.activation(erow[side][0:1, r0:r0 + rk],
                                     tps[0:1, 0:rk], Act.Exp)

            # ---- stage 2, split into head/tail parts ----
            # att[h] = (sum_r exp_r x[h,r]) / sum_r exp_r.  The unnormalized
            # partials only need the exp row, so the head part (all chunks of
            # the batch but the last) runs a chunk earlier than a normalized
            # formulation would allow; only the last chunk's sliver plus the
            # finalize remains on the critical tail.
            attps, atts = {}, {}

            def split_batch(side, b):
                # only the last-finishing batch of a side benefits from the
                # head/tail split; single-part elsewhere keeps the DVE
                # instruction count (and its fixed overheads) low
                return b == BLOC - 1

            def emit_part(side, b, head):
                sd = SD[side]
                rows = sd["rows"]
                b0, bend = b * rows, (b + 1) * rows
                if split_batch(side, b):
                    split = max(b0, sd["last"][b] * 128)
                    lo, hi = (b0, split) if head else (split, bend)
                else:
                    if head:
                        return
                    lo, hi = b0, bend
                if hi <= lo:
                    return
                w = hi - lo
                key = (side, b)
                if key not in attps:
                    attp = accpool.tile([128, HC * 2], f32,
                                        name=f"attp_{side}_{b}", tag="attp")
                    nc.vector.memset(attp[:, :], 0.0)
                    attps[key] = attp
                attp = attps[key]
                pi = 0 if head else 1
                abc = bpool.tile([128, w], f16, name=f"abc_{side}_{b}_{pi}",
                                 tag=f"abc_{int(head)}_{side}")
                nc.gpsimd.partition_broadcast(abc[:, :], erow[side][0:1, lo:hi])
                for hc in range(HC):
                    sj = jpool.tile([128, w], f16, name=f"sj_{side}_{b}_{hc}_{pi}",
                                    tag=f"sj_{side}")
                    nc.vector.scalar_tensor_tensor(
                        out=sj[:, :], in0=sd["xt"][:, hc, lo:hi],
                        scalar=1.0, in1=abc[:, :], op0=Alu.mult, op1=Alu.mult,
                        accum_out=attp[:, hc * 2 + pi:hc * 2 + pi + 1])

            def emit_finalize(side, b):
                sd = SD[side]
                rows = sd["rows"]
                b0 = b * rows
                ssum = spool.tile([1, 1], f32, name=f"ss_{side}_{b}", tag="ssum")
                nc.vector.tensor_reduce(out=ssum[0:1, 0:1],
                                        in_=erow[side][0:1, b0:b0 + rows],
                                        axis=Ax.X, op=Alu.add)
                rcp = spool.tile([1, 1], f32, name=f"rc_{side}_{b}", tag="rcp")
                nc.vector.reciprocal(rcp[0:1, 0:1], ssum[0:1, 0:1])
                rb = spool.tile([128, 1], f32, name=f"rb_{side}_{b}", tag="rb")
                nc.gpsimd.partition_broadcast(rb[:, 0:1], rcp[0:1, 0:1])
                attp = attps[(side, b)]
                attf = apool.tile([128, HC], f32, name=f"attf_{side}_{b}",
                                  tag="attf")
                nc.vector.tensor_reduce(
                    out=attf[:, :],
                    in_=attp[:, :].rearrange("p (h t) -> p h t", t=2),
                    axis=Ax.X, op=Alu.add)
                attsc = apool.tile([128, HC], f32, name=f"attsc_{side}_{b}",
                                   tag="attsc")
                nc.scalar.activation(attsc[:, :], attf[:, :], Act.Copy,
                                     scale=rb[:, 0:1])
                atts[(side, b)] = attsc

            def emit_attT(side, b):
                """PE transpose of the output column-tile + writeback."""
                att = atts[(side, b)]
                atp = tpool.tile([8, 128], f32, name=f"atp_{side}_{b}", tag="tp")
                nc.tensor.transpose(atp[0:8, 0:128], att[:, 0:HC],
                                    ident[:, :])
                osb = spool.tile([8, 128], f32, name=f"osb_{side}_{b}", tag="osb")
                nc.scalar.activation(osb[:, :], atp[:, :], Act.Copy)
                nc.sync.dma_start(out=out_d[SD[side]["oidx"], b], in_=osb[:, :])

            def after_T(side, ci):
                for b in range(BLOC):
                    if SD[side]["last"][b] - 1 == ci:
                        emit_part(side, b, head=True)
                    if SD[side]["last"][b] == ci:
                        emit_part(side, b, head=False)
                        emit_finalize(side, b)

            # ---- emission schedule ----
            # PE order: img prologue (c0-2, hc-major, DMA-paced), img c3-c5,
            # all dns chunks (their xt arrives while img computes), and the
            # 16-row img c6 last so the end-of-kernel chain is short.  Score
            # transposes are deferred >=1 chunk; output transposes >=2.
            emit_mm_prologue("img", [0, 1, 2])
            emit_mm("img", 3)
            for ci in (0, 1, 2):
                emit_T("img", ci); after_T("img", ci)
            emit_mm("img", 4); emit_T("img", 3); after_T("img", 3)
            emit_mm("img", 5); emit_T("img", 4); after_T("img", 4)
            emit_mm("dns", 0); emit_T("img", 5); after_T("img", 5)
            emit_mm("dns", 1); emit_attT("img", 0)
            emit_mm("dns", 2); emit_attT("img", 1); emit_T("dns", 0); after_T("dns", 0)
            emit_mm("dns", 3); emit_attT("img", 2); emit_T("dns", 1); after_T("dns", 1)
            attT_slot = {7: ("dns", 0), 11: ("dns", 1), 15: ("dns", 2)}
            for ci in range(4, 16):
                emit_mm("dns", ci)
                if ci in attT_slot:
                    emit_attT(*attT_slot[ci])
                emit_T("dns", ci - 2); after_T("dns", ci - 2)
            emit_mm("img", 6)
            emit_T("dns", 14); after_T("dns", 14)
            emit_T("dns", 15); after_T("dns", 15)
            emit_T("img", 6); after_T("img", 6)
            emit_attT("dns", 3)
            emit_attT("img", 3)

    nc.compile()
    return nc


def _get_nc():
    if "nc" not in _CACHE:
        _CACHE["nc"] = build_nc()
    return _CACHE["nc"]


def make_in_maps(inputs):
    dns = np.asarray(inputs["dns_feature"], dtype=np.float32)
    img = np.asarray(inputs["img_features"], dtype=np.float32)
    W_i1 = np.asarray(inputs["W_i1"], dtype=np.float32)
    W_d2 = np.asarray(inputs["W_d2"], dtype=np.float32)
    wB = np.asarray(inputs["w_att1"], dtype=np.float32)[H:]
    wD = np.asarray(inputs["w_att2"], dtype=np.float32)[H:]

    wt_i1 = np.ascontiguousarray(W_i1.T).reshape(HC, 128, H).astype(_BF16)
    wt_d2 = np.ascontiguousarray(W_d2.T).reshape(HC, 128, H).astype(_BF16)
    wr_b = np.ascontiguousarray(np.broadcast_to(wB, (128, H)).astype(_BF16))
    wr_d = np.ascontiguousarray(np.broadcast_to(wD, (128, H)).astype(_BF16))
    ident = np.eye(128, dtype=np.float32)

    in_maps = []
    for k in range(NCORES):
        sl = slice(k * BLOC, (k + 1) * BLOC)
        xd = np.ascontiguousarray(
            dns[sl].transpose(2, 0, 1).reshape(HC, 128, DTOT).astype(_BF16))
        xi = np.ascontiguousarray(
            img[sl].transpose(2, 0, 1).reshape(HC, 128, ITOT).astype(_BF16))
        in_maps.append({
            "xt_dns": xd, "xt_img": xi,
            "wt_i1": wt_i1, "wt_d2": wt_d2,
            "wrow_b": wr_b, "wrow_d": wr_d, "ident": ident,
        })
    return in_maps


def kernel(**inputs):
    from concourse.bass_utils import run_bass_kernel_spmd

    nc = _get_nc()
    in_maps = make_in_maps(inputs)
    res = run_bass_kernel_spmd(nc, in_maps, list(range(NCORES))).results
    out = np.stack([np.asarray(res[k]["out_all"]) for k in range(NCORES)])
    img_rows = out[:, 0].reshape(B, H)
    dns_rows = out[:, 1].reshape(B, H)
    att_dns = np.ascontiguousarray(
        np.broadcast_to(dns_rows[:, None, :], (B, S, H)))
    att_img = np.ascontiguousarray(
        np.broadcast_to(img_rows[:, None, :], (B, S, H)))
    return att_dns, att_img


# revision 19
# speedup vs baseline: 1.2514x; 1.0708x over previous
"""CoAttention ImageDNS kernel for Trainium2 (8 NeuronCores, Bass/Tile).

Math: the reference computes two additive-attention blocks. In both, the
softmax'd score is  score[b, q, k] = f(q-side)[b, q] + g(k-side)[b, k] + c,
and softmax over k is invariant to the q-dependent (and constant) terms, so
the attention weights are independent of the query index:

  visual_att[b, s, :]  = softmax_r( wB . tanh(W_i1 @ img[b, r]) )
  textual_att[b, i, :] = softmax_j( wD . tanh(W_d2 @ dns[b, j]) )

Hence both outputs are per-batch rank-1 broadcasts:

  att_img_features[b, s, :] = visual_att[b]  @ img[b]   (same for all s)
  att_dns_features[b, i, :] = textual_att[b] @ dns[b]   (same for all i)

W_d1/b_d1/w_att1[:H]/b_att1/W_i2/b_i2/w_att2[:H]/b_att2 cancel entirely.

Sharding: pure data-parallel over batch, 4 batches per core, no collectives.

Device dataflow (per core), designed around the bf16 PE streaming roofline
(~216 ns per K=128 N=512 matmul; LDWEIGHTS hides under the stream):
  - Only the h-transposed activations xt[h, row] are loaded (bf16); the rows
    of all 4 batches are packed along the free dim so row-chunks of 128 have
    no per-batch padding waste (784 img rows -> 7 chunks, 2048 dns -> 16).
  - Projection: chunk-major MMs, activations stationary, weights streaming.
  - score chain per chunk: tanh (ScalarE, bf16 out) -> scalar_tensor_tensor
    with the wB/wD broadcast row + free-dim accumulate (VectorE) giving the
    score column [rk, 1]; a PE transpose turns it into a score row; exp on
    ScalarE writes the per-side exp row [1, rows].
  - per batch: row-sum + reciprocal + normalize (VectorE), partition-
    broadcast of the normalized attention row (GpSimd), then stage-2 as 8
    STT free-dim-accumulate ops over xt (VectorE) - no xn loads, no PE.
  - outputs: one [H] vector per (batch, side), PE-transposed to [8, 128]
    and DMA'd out (32 KB total instead of 16.8 MB of broadcast rows); the
    host broadcasts to the full (B, S, H) shape during unshard.
  - PE-queue ops that depend on the VectorE chain (the transposes) are
    emitted 1-2 chunks late so the in-order PE queue never stalls.
"""

import sys
import numpy as np
import ml_dtypes

_BF16 = ml_dtypes.bfloat16

for _p in ("/opt/trn_rl_repo", "/root/.axon_site/_ro/trn_rl_repo"):
    if _p not in sys.path:
        sys.path.append(_p)

B, S, R, H = 32, 512, 196, 1024
NCORES = 8
BLOC = B // NCORES          # batches per core
OC = 512                    # output-chunk (one fp32 PSUM bank)
HC = H // 128               # contraction chunks
DTOT = BLOC * S             # packed dns rows per core (2048)
ITOT = BLOC * R             # packed img rows per core (784)

_CACHE = {}


def _row_chunks(n):
    out, o = [], 0
    while o < n:
        out.append((o, min(128, n - o)))
        o += 128
    return out


def build_nc():
    from concourse import bacc, mybir
    from concourse import tile

    f32, f16 = mybir.dt.float32, mybir.dt.bfloat16
    Act = mybir.ActivationFunctionType
    Alu = mybir.AluOpType
    Ax = mybir.AxisListType

    nc = bacc.Bacc("TRN2", target_bir_lowering=False, debug=False)

    xt_dns_d = nc.dram_tensor("xt_dns", [HC, 128, DTOT], f16, kind="ExternalInput")
    xt_img_d = nc.dram_tensor("xt_img", [HC, 128, ITOT], f16, kind="ExternalInput")
    wt_i1_d = nc.dram_tensor("wt_i1", [HC, 128, H], f16, kind="ExternalInput")
    wt_d2_d = nc.dram_tensor("wt_d2", [HC, 128, H], f16, kind="ExternalInput")
    wr_b_d = nc.dram_tensor("wrow_b", [128, H], f16, kind="ExternalInput")
    wr_d_d = nc.dram_tensor("wrow_d", [128, H], f16, kind="ExternalInput")
    ident_d = nc.dram_tensor("ident", [128, 128], f32, kind="ExternalInput")
    out_d = nc.dram_tensor("out_all", [2, BLOC, HC, 128], f32, kind="ExternalOutput")

    IMG_RCS = _row_chunks(ITOT)      # 7 chunks (6x128 + 16)
    DNS_RCS = _row_chunks(DTOT)      # 16 chunks
    # batch -> last chunk holding its rows (chunk boundaries don't align with
    # batch boundaries on the img side)
    img_last_chunk = [((b + 1) * R - 1) // 128 for b in range(BLOC)]
    dns_last_chunk = [((b + 1) * S - 1) // 128 for b in range(BLOC)]

    with tile.TileContext(nc) as tc:
        with (
            tc.tile_pool(name="const", bufs=1) as cpool,
            tc.tile_pool(name="th", bufs=3) as thpool,
            tc.tile_pool(name="scr", bufs=2) as jpool,
            tc.tile_pool(name="small", bufs=3) as spool,
            tc.tile_pool(name="attp", bufs=4) as apool,
            tc.tile_pool(name="attacc", bufs=8) as accpool,
            tc.tile_pool(name="bc", bufs=2) as bpool,
            tc.tile_pool(name="pp", bufs=3, space="PSUM") as ppool,
            tc.tile_pool(name="tp", bufs=2, space="PSUM") as tpool,
        ):
            # ---- persistent SBUF tiles ----
            xt_img = cpool.tile([128, HC, ITOT], f16, name="xt_img_sb")
            xt_dns = cpool.tile([128, HC, DTOT], f16, name="xt_dns_sb")
            wt_img = cpool.tile([128, HC, H], f16, name="wt_img_sb")
            wt_dns = cpool.tile([128, HC, H], f16, name="wt_dns_sb")
            wrb = {"img": cpool.tile([128, H], f16, name="wrb_img"),
                   "dns": cpool.tile([128, H], f16, name="wrb_dns")}
            ident = cpool.tile([128, 128], f32, name="ident_sb")
            # exp rows in bf16: keeps every tensor operand of the stage-2
            # STTs 16-bit (2x DVE mode) at ~0.4% relative cost on the weights
            erow = {"img": cpool.tile([1, ITOT], f16, name="erow_img"),
                    "dns": cpool.tile([1, DTOT], f16, name="erow_dns")}

            SD = {
                "img": dict(xt=xt_img, wt=wt_img, rcs=IMG_RCS, rows=R,
                            last=img_last_chunk, oidx=0),
                "dns": dict(xt=xt_dns, wt=wt_dns, rcs=DNS_RCS, rows=S,
                            last=dns_last_chunk, oidx=1),
            }

            # ---- PE warmup: ~10 dummy matmuls on scratch data get the HAM
            # clock gate to 8/8 (2.4 GHz) during the first ~4us, which is
            # DMA-bound anyway; real matmuls then run warm from the start.
            warm_sb = cpool.tile([128, OC], f16, name="warm_sb")
            nc.vector.memset(warm_sb[:, :], 0.0)
            warm_ps = ppool.tile([128, H], f32, name="warm_ps", tag="pp")
            for i in range(10):
                nc.tensor.matmul(warm_ps[:, 0:OC], lhsT=warm_sb[:, 0:128],
                                 rhs=warm_sb[:, :], start=True, stop=True)

            # ---- input DMAs ----
            # three issue queues in parallel from t=0:
            #   sync:   img xt per-hc, then dns xt in two column halves
            #   gpsimd: img wt first (nothing queued ahead), then dns wt
            #   scalar: ident + wB/wD rows (small; needed only at ~10us)
            nc.scalar.dma_start(out=ident[:, :], in_=ident_d[:, :])
            nc.scalar.dma_start(out=wrb["img"][:, :], in_=wr_b_d[:, :])
            nc.scalar.dma_start(out=wrb["dns"][:, :], in_=wr_d_d[:, :])
            for hc in range(HC):
                nc.gpsimd.dma_start(out=wt_img[:, hc, :], in_=wt_i1_d[hc])
                nc.sync.dma_start(out=xt_img[:, hc, :], in_=xt_img_d[hc])
            for hc in range(HC):
                nc.gpsimd.dma_start(out=wt_dns[:, hc, :], in_=wt_d2_d[hc])
            HD = DTOT // 2
            for half in range(2):
                cs = slice(half * HD, (half + 1) * HD)
                for hc in range(HC):
                    nc.sync.dma_start(out=xt_dns[:, hc, cs],
                                      in_=xt_dns_d[hc][:, cs])

            # ---- per-chunk pieces ----
            tcols = {}
            tps_tiles = {}

            def emit_mm(side, ci, split_chain=False):
                """proj MMs for one chunk + its (non-PE) score chain.
                split_chain: oc-major MM order + per-half tanh/STT so most of
                the score chain overlaps the second half's matmuls — used for
                the final chunk of each side to shorten the kernel tail."""
                sd = SD[side]
                r0, rk = sd["rcs"][ci]
                ps = ppool.tile([128, H], f32, name=f"ps_{side}_{ci}", tag="pp")
                if not split_chain:
                    for hc in range(HC):
                        lhs = sd["xt"][:, hc, r0:r0 + rk]
                        for oc in range(2):
                            nc.tensor.matmul(
                                ps[0:rk, oc * OC:(oc + 1) * OC],
                                lhsT=lhs,
                                rhs=sd["wt"][:, hc, oc * OC:(oc + 1) * OC],
                                start=(hc == 0), stop=(hc == HC - 1))
                    emit_chain(side, ci, ps)
                    return
                halves = []
                for oc in range(2):
                    for hc in range(HC):
                        nc.tensor.matmul(
                            ps[0:rk, oc * OC:(oc + 1) * OC],
                            lhsT=sd["xt"][:, hc, r0:r0 + rk],
                            rhs=sd["wt"][:, hc, oc * OC:(oc + 1) * OC],
                            start=(hc == 0), stop=(hc == HC - 1))
                    th = thpool.tile([128, OC], f16,
                                     name=f"th_{side}_{ci}_{oc}", tag="thh")
                    nc.scalar.activation(th[0:rk, :],
                                         ps[0:rk, oc * OC:(oc + 1) * OC],
                                         Act.Tanh)
                    scr = jpool.tile([128, OC], f16,
                                     name=f"scr_{side}_{ci}_{oc}", tag="scrh")
                    tch = spool.tile([128, 1], f32,
                                     name=f"tch_{side}_{ci}_{oc}", tag="tcolh")
                    nc.vector.scalar_tensor_tensor(
                        out=scr[0:rk, :], in0=th[0:rk, :], scalar=1.0,
                        in1=wrb[side][0:rk, oc * OC:(oc + 1) * OC],
                        op0=Alu.mult, op1=Alu.mult, accum_out=tch[0:rk, :])
                    halves.append(tch)
                tcol = spool.tile([128, 1], f32, name=f"tc_{side}_{ci}",
                                  tag="tcol")
                nc.vector.scalar_tensor_tensor(
                    out=tcol[0:rk, :], in0=halves[0][0:rk, :], scalar=1.0,
                    in1=halves[1][0:rk, :], op0=Alu.mult, op1=Alu.add)
                tcols[(side, ci)] = tcol

            def emit_mm_prologue(side, cis):
                """hc-major MMs over several chunks: consumes the per-hc input
                DMAs progressively so the PE starts ~1.5us into the kernel."""
                sd = SD[side]
                pss = {}
                for ci in cis:
                    pss[ci] = ppool.tile([128, H], f32, name=f"ps_{side}_{ci}",
                                         tag="pp")
                for hc in range(HC):
                    for ci in cis:
                        r0, rk = sd["rcs"][ci]
                        lhs = sd["xt"][:, hc, r0:r0 + rk]
                        for oc in range(2):
                            nc.tensor.matmul(
                                pss[ci][0:rk, oc * OC:(oc + 1) * OC],
                                lhsT=lhs,
                                rhs=sd["wt"][:, hc, oc * OC:(oc + 1) * OC],
                                start=(hc == 0), stop=(hc == HC - 1))
                for ci in cis:
                    emit_chain(side, ci, pss[ci])

            def emit_chain(side, ci, ps):
                """tanh -> weighted free-dim reduce -> score column [rk, 1]."""
                sd = SD[side]
                r0, rk = sd["rcs"][ci]
                th = thpool.tile([128, H], f16, name=f"th_{side}_{ci}", tag="th")
                nc.scalar.activation(th[0:rk, :], ps[0:rk, :], Act.Tanh)
                scr = jpool.tile([128, H], f16, name=f"scr_{side}_{ci}", tag="scr")
                tcol = spool.tile([128, 1], f32, name=f"tc_{side}_{ci}", tag="tcol")
                nc.vector.scalar_tensor_tensor(
                    out=scr[0:rk, :], in0=th[0:rk, :], scalar=1.0,
                    in1=wrb[side][0:rk, :], op0=Alu.mult, op1=Alu.mult,
                    accum_out=tcol[0:rk, :])
                tcols[(side, ci)] = tcol

            def emit_T(side, ci):
                """PE transpose of the score column -> exp row slice.
                Emitted >=1 chunk after emit_mm so the PE queue never waits
                on the VectorE chain."""
                sd = SD[side]
                r0, rk = sd["rcs"][ci]
                tcol = tcols[(side, ci)]
                tps = tpool.tile([8, 128], f32, name=f"tps_{side}_{ci}", tag="tp")
                nc.tensor.transpose(tps[0:1, 0:rk], tcol[0:rk, 0:1],
                                    ident[0:rk, 0:rk])
                nc.scalar.activation(erow[side][0:1, r0:r0 + rk],
                                     tps[0:1, 0:rk], Act.Exp)

            # ---- stage 2, split into head/tail parts ----
            # att[h] = (sum_r exp_r x[h,r]) / sum_r exp_r.  The unnormalized
            # partials only need the exp row, so the head part (all chunks of
            # the batch but the last) runs a chunk earlier than a normalized
            # formulation would allow; only the last chunk's sliver plus the
            # finalize remains on the critical tail.
            attps, atts = {}, {}

            def split_batch(side, b):
                # only the last-finishing batch of a side benefits from the
                # head/tail split; single-part elsewhere keeps the DVE
                # instruction count (and its fixed overheads) low
                return b == BLOC - 1

            def emit_part(side, b, head):
                sd = SD[side]
                rows = sd["rows"]
                b0, bend = b * rows, (b + 1) * rows
                if split_batch(side, b):
                    split = max(b0, sd["last"][b] * 128)
                    lo, hi = (b0, split) if head else (split, bend)
                else:
                    if head:
                        return
                    lo, hi = b0, bend
                if hi <= lo:
                    return
                w = hi - lo
                key = (side, b)
                if key not in attps:
                    attp = accpool.tile([128, HC * 2], f32,
                                        name=f"attp_{side}_{b}", tag="attp")
                    nc.vector.memset(attp[:, :], 0.0)
                    attps[key] = attp
                attp = attps[key]
                pi = 0 if head else 1
                abc = bpool.tile([128, w], f16, name=f"abc_{side}_{b}_{pi}",
                                 tag=f"abc_{int(head)}_{side}")
                nc.gpsimd.partition_broadcast(abc[:, :], erow[side][0:1, lo:hi])
                for hc in range(HC):
                    sj = jpool.tile([128, w], f16, name=f"sj_{side}_{b}_{hc}_{pi}",
                                    tag=f"sj_{side}")
                    nc.vector.scalar_tensor_tensor(
                        out=sj[:, :], in0=sd["xt"][:, hc, lo:hi],
                        scalar=1.0, in1=abc[:, :], op0=Alu.mult, op1=Alu.mult,
                        accum_out=attp[:, hc * 2 + pi:hc * 2 + pi + 1])

            def emit_finalize(side, b):
                sd = SD[side]
                rows = sd["rows"]
                b0 = b * rows
                ssum = spool.tile([1, 1], f32, name=f"ss_{side}_{b}", tag="ssum")
                nc.vector.tensor_reduce(out=ssum[0:1, 0:1],
                                        in_=erow[side][0:1, b0:b0 + rows],
                                        axis=Ax.X, op=Alu.add)
                rcp = spool.tile([1, 1], f32, name=f"rc_{side}_{b}", tag="rcp")
                nc.vector.reciprocal(rcp[0:1, 0:1], ssum[0:1, 0:1])
                rb = spool.tile([128, 1], f32, name=f"rb_{side}_{b}", tag="rb")
                nc.gpsimd.partition_broadcast(rb[:, 0:1], rcp[0:1, 0:1])
                attp = attps[(side, b)]
                attf = apool.tile([128, HC], f32, name=f"attf_{side}_{b}",
                                  tag="attf")
                nc.vector.tensor_reduce(
                    out=attf[:, :],
                    in_=attp[:, :].rearrange("p (h t) -> p h t", t=2),
                    axis=Ax.X, op=Alu.add)
                attsc = apool.tile([128, HC], f32, name=f"attsc_{side}_{b}",
                                   tag="attsc")
                nc.scalar.activation(attsc[:, :], attf[:, :], Act.Copy,
                                     scale=rb[:, 0:1])
                atts[(side, b)] = attsc

            def emit_attT(side, b):
                """PE transpose of the output column-tile + writeback."""
                att = atts[(side, b)]
                atp = tpool.tile([8, 128], f32, name=f"atp_{side}_{b}", tag="tp")
                nc.tensor.transpose(atp[0:8, 0:128], att[:, 0:HC],
                                    ident[:, :])
                osb = spool.tile([8, 128], f32, name=f"osb_{side}_{b}", tag="osb")
                nc.scalar.activation(osb[:, :], atp[:, :], Act.Copy)
                nc.sync.dma_start(out=out_d[SD[side]["oidx"], b], in_=osb[:, :])

            def after_T(side, ci):
                for b in range(BLOC):
                    if SD[side]["last"][b] - 1 == ci:
                        emit_part(side, b, head=True)
                    if SD[side]["last"][b] == ci:
                        emit_part(side, b, head=False)
                        emit_finalize(side, b)

            # ---- emission schedule ----
            # PE order: img prologue (c0-2, hc-major, DMA-paced), img c3-c5,
            # all dns chunks (their xt arrives while img computes), and the
            # 16-row img c6 last so the end-of-kernel chain is short.  Score
            # transposes are deferred >=1 chunk; output transposes >=2.
            emit_mm_prologue("img", [0, 1, 2])
            emit_mm("img", 3)
            for ci in (0, 1, 2):
                emit_T("img", ci); after_T("img", ci)
            emit_mm("img", 4); emit_T("img", 3); after_T("img", 3)
            emit_mm("img", 5); emit_T("img", 4); after_T("img", 4)
            emit_mm("dns", 0); emit_T("img", 5); after_T("img", 5)
            emit_mm("dns", 1); emit_attT("img", 0)
            emit_mm("dns", 2); emit_attT("img", 1); emit_T("dns", 0); after_T("dns", 0)
            emit_mm("dns", 3); emit_attT("img", 2); emit_T("dns", 1); after_T("dns", 1)
            attT_slot = {7: ("dns", 0), 11: ("dns", 1), 15: ("dns", 2)}
            for ci in range(4, 15):
                emit_mm("dns", ci)
                if ci in attT_slot:
                    emit_attT(*attT_slot[ci])
                emit_T("dns", ci - 2); after_T("dns", ci - 2)
            emit_mm("dns", 15, split_chain=True)
            emit_T("dns", 13); after_T("dns", 13)
            emit_T("dns", 14); after_T("dns", 14)
            emit_mm("img", 6, split_chain=True)
            emit_attT(*attT_slot[15])
            emit_T("dns", 15); after_T("dns", 15)
            emit_T("img", 6); after_T("img", 6)
            emit_attT("dns", 3)
            emit_attT("img", 3)

    nc.compile()
    return nc


def _get_nc():
    if "nc" not in _CACHE:
        _CACHE["nc"] = build_nc()
    return _CACHE["nc"]


def make_in_maps(inputs):
    dns = np.asarray(inputs["dns_feature"], dtype=np.float32)
    img = np.asarray(inputs["img_features"], dtype=np.float32)
    W_i1 = np.asarray(inputs["W_i1"], dtype=np.float32)
    W_d2 = np.asarray(inputs["W_d2"], dtype=np.float32)
    wB = np.asarray(inputs["w_att1"], dtype=np.float32)[H:]
    wD = np.asarray(inputs["w_att2"], dtype=np.float32)[H:]

    wt_i1 = np.ascontiguousarray(W_i1.T).reshape(HC, 128, H).astype(_BF16)
    wt_d2 = np.ascontiguousarray(W_d2.T).reshape(HC, 128, H).astype(_BF16)
    wr_b = np.ascontiguousarray(np.broadcast_to(wB, (128, H)).astype(_BF16))
    wr_d = np.ascontiguousarray(np.broadcast_to(wD, (128, H)).astype(_BF16))
    ident = np.eye(128, dtype=np.float32)

    in_maps = []
    for k in range(NCORES):
        sl = slice(k * BLOC, (k + 1) * BLOC)
        xd = np.ascontiguousarray(
            dns[sl].transpose(2, 0, 1).reshape(HC, 128, DTOT).astype(_BF16))
        xi = np.ascontiguousarray(
            img[sl].transpose(2, 0, 1).reshape(HC, 128, ITOT).astype(_BF16))
        in_maps.append({
            "xt_dns": xd, "xt_img": xi,
            "wt_i1": wt_i1, "wt_d2": wt_d2,
            "wrow_b": wr_b, "wrow_d": wr_d, "ident": ident,
        })
    return in_maps


def kernel(**inputs):
    from concourse.bass_utils import run_bass_kernel_spmd

    nc = _get_nc()
    in_maps = make_in_maps(inputs)
    res = run_bass_kernel_spmd(nc, in_maps, list(range(NCORES))).results
    out = np.stack([np.asarray(res[k]["out_all"]) for k in range(NCORES)])
    img_rows = out[:, 0].reshape(B, H)
    dns_rows = out[:, 1].reshape(B, H)
    att_dns = np.ascontiguousarray(
        np.broadcast_to(dns_rows[:, None, :], (B, S, H)))
    att_img = np.ascontiguousarray(
        np.broadcast_to(img_rows[:, None, :], (B, S, H)))
    return att_dns, att_img


# revision 22
# speedup vs baseline: 1.3099x; 1.0467x over previous
"""CoAttention ImageDNS kernel for Trainium2 (8 NeuronCores, Bass/Tile).

Math: the reference computes two additive-attention blocks. In both, the
softmax'd score is  score[b, q, k] = f(q-side)[b, q] + g(k-side)[b, k] + c,
and softmax over k is invariant to the q-dependent (and constant) terms, so
the attention weights are independent of the query index:

  visual_att[b, s, :]  = softmax_r( wB . tanh(W_i1 @ img[b, r]) )
  textual_att[b, i, :] = softmax_j( wD . tanh(W_d2 @ dns[b, j]) )

Hence both outputs are per-batch rank-1 broadcasts:

  att_img_features[b, s, :] = visual_att[b]  @ img[b]   (same for all s)
  att_dns_features[b, i, :] = textual_att[b] @ dns[b]   (same for all i)

W_d1/b_d1/w_att1[:H]/b_att1/W_i2/b_i2/w_att2[:H]/b_att2 cancel entirely.

Sharding: pure data-parallel over batch, 4 batches per core, no collectives.

Device dataflow (per core), designed around the bf16 PE streaming roofline
(~216 ns per K=128 N=512 matmul; LDWEIGHTS hides under the stream):
  - Only the h-transposed activations xt[h, row] are loaded (bf16); the rows
    of all 4 batches are packed along the free dim so row-chunks of 128 have
    no per-batch padding waste (784 img rows -> 7 chunks, 2048 dns -> 16).
  - Projection: chunk-major MMs, activations stationary, weights streaming.
  - score chain per chunk: tanh (ScalarE, bf16 out) -> scalar_tensor_tensor
    with the wB/wD broadcast row + free-dim accumulate (VectorE) giving the
    score column [rk, 1]; a PE transpose turns it into a score row; exp on
    ScalarE writes the per-side exp row [1, rows].
  - per batch: row-sum + reciprocal + normalize (VectorE), partition-
    broadcast of the normalized attention row (GpSimd), then stage-2 as 8
    STT free-dim-accumulate ops over xt (VectorE) - no xn loads, no PE.
  - outputs: one [H] vector per (batch, side), PE-transposed to [8, 128]
    and DMA'd out (32 KB total instead of 16.8 MB of broadcast rows); the
    host broadcasts to the full (B, S, H) shape during unshard.
  - PE-queue ops that depend on the VectorE chain (the transposes) are
    emitted 1-2 chunks late so the in-order PE queue never stalls.
"""

import sys
import numpy as np
import ml_dtypes

_BF16 = ml_dtypes.bfloat16

for _p in ("/opt/trn_rl_repo", "/root/.axon_site/_ro/trn_rl_repo"):
    if _p not in sys.path:
        sys.path.append(_p)

B, S, R, H = 32, 512, 196, 1024
NCORES = 8
BLOC = B // NCORES          # batches per core
OC = 512                    # output-chunk (one fp32 PSUM bank)
HC = H // 128               # contraction chunks
DTOT = BLOC * S             # packed dns rows per core (2048)
ITOT = BLOC * R             # packed img rows per core (784)

_CACHE = {}


def _row_chunks(n):
    out, o = [], 0
    while o < n:
        out.append((o, min(128, n - o)))
        o += 128
    return out


def build_nc():
    from concourse import bacc, mybir
    from concourse import tile

    f32, f16 = mybir.dt.float32, mybir.dt.bfloat16
    Act = mybir.ActivationFunctionType
    Alu = mybir.AluOpType
    Ax = mybir.AxisListType

    nc = bacc.Bacc("TRN2", target_bir_lowering=False, debug=False)

    xt_dns_d = nc.dram_tensor("xt_dns", [HC, 128, DTOT], f16, kind="ExternalInput")
    xt_img_d = nc.dram_tensor("xt_img", [HC, 128, ITOT], f16, kind="ExternalInput")
    wt_i1_d = nc.dram_tensor("wt_i1", [HC, 128, H], f16, kind="ExternalInput")
    wt_d2_d = nc.dram_tensor("wt_d2", [HC, 128, H], f16, kind="ExternalInput")
    wr_b_d = nc.dram_tensor("wrow_b", [128, H], f16, kind="ExternalInput")
    wr_d_d = nc.dram_tensor("wrow_d", [128, H], f16, kind="ExternalInput")
    ident_d = nc.dram_tensor("ident", [128, 128], f32, kind="ExternalInput")
    out_d = nc.dram_tensor("out_all", [2, BLOC, HC, 128], f32, kind="ExternalOutput")

    IMG_RCS = _row_chunks(ITOT)      # 7 chunks (6x128 + 16)
    DNS_RCS = _row_chunks(DTOT)      # 16 chunks
    # batch -> last chunk holding its rows (chunk boundaries don't align with
    # batch boundaries on the img side)
    img_last_chunk = [((b + 1) * R - 1) // 128 for b in range(BLOC)]
    dns_last_chunk = [((b + 1) * S - 1) // 128 for b in range(BLOC)]

    with tile.TileContext(nc) as tc:
        with (
            tc.tile_pool(name="const", bufs=1) as cpool,
            tc.tile_pool(name="th", bufs=3) as thpool,
            tc.tile_pool(name="scr", bufs=2) as jpool,
            tc.tile_pool(name="small", bufs=3) as spool,
            tc.tile_pool(name="attp", bufs=4) as apool,
            tc.tile_pool(name="attacc", bufs=8) as accpool,
            tc.tile_pool(name="bc", bufs=2) as bpool,
            tc.tile_pool(name="pp", bufs=3, space="PSUM") as ppool,
            tc.tile_pool(name="tp", bufs=2, space="PSUM") as tpool,
        ):
            # ---- persistent SBUF tiles ----
            xt_img = cpool.tile([128, HC, ITOT], f16, name="xt_img_sb")
            xt_dns = cpool.tile([128, HC, DTOT], f16, name="xt_dns_sb")
            wt_img = cpool.tile([128, HC, H], f16, name="wt_img_sb")
            wt_dns = cpool.tile([128, HC, H], f16, name="wt_dns_sb")
            wrb = {"img": cpool.tile([128, H], f16, name="wrb_img"),
                   "dns": cpool.tile([128, H], f16, name="wrb_dns")}
            ident = cpool.tile([128, 128], f32, name="ident_sb")
            # exp rows in bf16: keeps every tensor operand of the stage-2
            # STTs 16-bit (2x DVE mode) at ~0.4% relative cost on the weights
            erow = {"img": cpool.tile([1, ITOT], f16, name="erow_img"),
                    "dns": cpool.tile([1, DTOT], f16, name="erow_dns")}

            SD = {
                "img": dict(xt=xt_img, wt=wt_img, rcs=IMG_RCS, rows=R,
                            last=img_last_chunk, oidx=0),
                "dns": dict(xt=xt_dns, wt=wt_dns, rcs=DNS_RCS, rows=S,
                            last=dns_last_chunk, oidx=1),
            }

            # ---- PE warmup: ~10 dummy matmuls on scratch data get the HAM
            # clock gate to 8/8 (2.4 GHz) during the first ~4us, which is
            # DMA-bound anyway; real matmuls then run warm from the start.
            warm_sb = cpool.tile([128, OC], f16, name="warm_sb")
            nc.vector.memset(warm_sb[:, :], 0.0)
            warm_ps = ppool.tile([128, H], f32, name="warm_ps", tag="pp")
            for i in range(10):
                nc.tensor.matmul(warm_ps[:, 0:OC], lhsT=warm_sb[:, 0:128],
                                 rhs=warm_sb[:, :], start=True, stop=True)

            # ---- input DMAs ----
            # three issue queues in parallel from t=0:
            #   sync:   img xt per-hc, then dns xt in two column halves
            #   gpsimd: img wt first (nothing queued ahead), then dns wt
            #   scalar: ident + wB/wD rows (small; needed only at ~10us)
            nc.scalar.dma_start(out=ident[:, :], in_=ident_d[:, :])
            nc.scalar.dma_start(out=wrb["img"][:, :], in_=wr_b_d[:, :])
            nc.scalar.dma_start(out=wrb["dns"][:, :], in_=wr_d_d[:, :])
            for hc in range(HC):
                nc.gpsimd.dma_start(out=wt_img[:, hc, :], in_=wt_i1_d[hc])
                nc.sync.dma_start(out=xt_img[:, hc, :], in_=xt_img_d[hc])
            for hc in range(HC):
                nc.gpsimd.dma_start(out=wt_dns[:, hc, :], in_=wt_d2_d[hc])
            HD = DTOT // 2
            for half in range(2):
                cs = slice(half * HD, (half + 1) * HD)
                for hc in range(HC):
                    nc.sync.dma_start(out=xt_dns[:, hc, cs],
                                      in_=xt_dns_d[hc][:, cs])

            # ---- per-chunk pieces ----
            tcols = {}
            tps_tiles = {}

            def emit_mm(side, ci, split_chain=False):
                """proj MMs for one chunk + its (non-PE) score chain.
                split_chain: oc-major MM order + per-half tanh/STT so most of
                the score chain overlaps the second half's matmuls — used for
                the final chunk of each side to shorten the kernel tail."""
                sd = SD[side]
                r0, rk = sd["rcs"][ci]
                ps = ppool.tile([128, H], f32, name=f"ps_{side}_{ci}", tag="pp")
                if not split_chain:
                    for hc in range(HC):
                        lhs = sd["xt"][:, hc, r0:r0 + rk]
                        for oc in range(2):
                            nc.tensor.matmul(
                                ps[0:rk, oc * OC:(oc + 1) * OC],
                                lhsT=lhs,
                                rhs=sd["wt"][:, hc, oc * OC:(oc + 1) * OC],
                                start=(hc == 0), stop=(hc == HC - 1))
                    emit_chain(side, ci, ps)
                    return
                halves = []
                for oc in range(2):
                    for hc in range(HC):
                        nc.tensor.matmul(
                            ps[0:rk, oc * OC:(oc + 1) * OC],
                            lhsT=sd["xt"][:, hc, r0:r0 + rk],
                            rhs=sd["wt"][:, hc, oc * OC:(oc + 1) * OC],
                            start=(hc == 0), stop=(hc == HC - 1))
                    th = thpool.tile([128, OC], f16,
                                     name=f"th_{side}_{ci}_{oc}", tag="thh")
                    nc.scalar.activation(th[0:rk, :],
                                         ps[0:rk, oc * OC:(oc + 1) * OC],
                                         Act.Tanh)
                    scr = jpool.tile([128, OC], f16,
                                     name=f"scr_{side}_{ci}_{oc}", tag="scrh")
                    tch = spool.tile([128, 1], f32,
                                     name=f"tch_{side}_{ci}_{oc}", tag="tcolh")
                    nc.vector.scalar_tensor_tensor(
                        out=scr[0:rk, :], in0=th[0:rk, :], scalar=1.0,
                        in1=wrb[side][0:rk, oc * OC:(oc + 1) * OC],
                        op0=Alu.mult, op1=Alu.mult, accum_out=tch[0:rk, :])
                    halves.append(tch)
                tcol = spool.tile([128, 1], f32, name=f"tc_{side}_{ci}",
                                  tag="tcol")
                nc.vector.scalar_tensor_tensor(
                    out=tcol[0:rk, :], in0=halves[0][0:rk, :], scalar=1.0,
                    in1=halves[1][0:rk, :], op0=Alu.mult, op1=Alu.add)
                tcols[(side, ci)] = tcol

            def emit_mm_prologue(side, cis):
                """hc-major MMs over several chunks: consumes the per-hc input
                DMAs progressively so the PE starts ~1.5us into the kernel."""
                sd = SD[side]
                pss = {}
                for ci in cis:
                    pss[ci] = ppool.tile([128, H], f32, name=f"ps_{side}_{ci}",
                                         tag="pp")
                for hc in range(HC):
                    for ci in cis:
                        r0, rk = sd["rcs"][ci]
                        lhs = sd["xt"][:, hc, r0:r0 + rk]
                        for oc in range(2):
                            nc.tensor.matmul(
                                pss[ci][0:rk, oc * OC:(oc + 1) * OC],
                                lhsT=lhs,
                                rhs=sd["wt"][:, hc, oc * OC:(oc + 1) * OC],
                                start=(hc == 0), stop=(hc == HC - 1))
                for ci in cis:
                    emit_chain(side, ci, pss[ci])

            def emit_chain(side, ci, ps):
                """tanh -> weighted free-dim reduce -> score column [rk, 1]."""
                sd = SD[side]
                r0, rk = sd["rcs"][ci]
                th = thpool.tile([128, H], f16, name=f"th_{side}_{ci}", tag="th")
                nc.scalar.activation(th[0:rk, :], ps[0:rk, :], Act.Tanh)
                scr = jpool.tile([128, H], f16, name=f"scr_{side}_{ci}", tag="scr")
                tcol = spool.tile([128, 1], f32, name=f"tc_{side}_{ci}", tag="tcol")
                nc.vector.scalar_tensor_tensor(
                    out=scr[0:rk, :], in0=th[0:rk, :], scalar=1.0,
                    in1=wrb[side][0:rk, :], op0=Alu.mult, op1=Alu.mult,
                    accum_out=tcol[0:rk, :])
                tcols[(side, ci)] = tcol

            def emit_T(side, ci):
                """PE transpose of the score column -> exp row slice.
                Emitted >=1 chunk after emit_mm so the PE queue never waits
                on the VectorE chain."""
                sd = SD[side]
                r0, rk = sd["rcs"][ci]
                tcol = tcols[(side, ci)]
                tps = tpool.tile([8, 128], f32, name=f"tps_{side}_{ci}", tag="tp")
                nc.tensor.transpose(tps[0:1, 0:rk], tcol[0:rk, 0:1],
                                    ident[0:rk, 0:rk])
                nc.scalar.activation(erow[side][0:1, r0:r0 + rk],
                                     tps[0:1, 0:rk], Act.Exp)

            # ---- stage 2, split into head/tail parts ----
            # att[h] = (sum_r exp_r x[h,r]) / sum_r exp_r.  The unnormalized
            # partials only need the exp row, so the head part (all chunks of
            # the batch but the last) runs a chunk earlier than a normalized
            # formulation would allow; only the last chunk's sliver plus the
            # finalize remains on the critical tail.
            attps, atts = {}, {}
            NPART = 4

            # part plan: (side, b) -> list of (lo, hi, ready_chunk).  Each
            # part becomes one broadcast + 8 STT accumulates, emitted at
            # after_T(side, ready_chunk).  Bounding every part under ~2.6us
            # of DVE time keeps the in-order DVE queue from ever delaying
            # the next chunk's score chain (one PE chunk = ~3.5us).
            part_plan = {}
            for b in range(BLOC):
                b0, bend = b * R, (b + 1) * R
                lc = img_last_chunk[b]
                if b == BLOC - 1:
                    part_plan[("img", b)] = [(b0, lc * 128, lc - 1),
                                             (lc * 128, bend, lc)]
                else:
                    part_plan[("img", b)] = [(b0, bend, lc)]
            for b in range(BLOC):
                b0, bend = b * S, (b + 1) * S
                c0 = 4 * b
                if b == BLOC - 1:
                    part_plan[("dns", b)] = [(b0, b0 + 256, c0 + 1),
                                             (b0 + 256, b0 + 384, c0 + 2),
                                             (b0 + 384, bend, c0 + 3)]
                else:
                    part_plan[("dns", b)] = [(b0, b0 + 256, c0 + 1),
                                             (b0 + 256, bend, c0 + 3)]

            def emit_part(side, b, lo, hi, pi):
                sd = SD[side]
                w = hi - lo
                key = (side, b)
                if key not in attps:
                    attp = accpool.tile([128, HC * NPART], f32,
                                        name=f"attp_{side}_{b}", tag="attp")
                    nc.vector.memset(attp[:, :], 0.0)
                    attps[key] = attp
                attp = attps[key]
                abc = bpool.tile([128, w], f16, name=f"abc_{side}_{b}_{pi}",
                                 tag=f"abc_{side}_{pi}")
                nc.gpsimd.partition_broadcast(abc[:, :], erow[side][0:1, lo:hi])
                for hc in range(HC):
                    sj = jpool.tile([128, w], f16, name=f"sj_{side}_{b}_{hc}_{pi}",
                                    tag=f"sj_{side}")
                    nc.vector.scalar_tensor_tensor(
                        out=sj[:, :], in0=sd["xt"][:, hc, lo:hi],
                        scalar=1.0, in1=abc[:, :], op0=Alu.mult, op1=Alu.mult,
                        accum_out=attp[:, hc * NPART + pi:hc * NPART + pi + 1])

            def emit_finalize(side, b):
                sd = SD[side]
                rows = sd["rows"]
                b0 = b * rows
                ssum = spool.tile([1, 1], f32, name=f"ss_{side}_{b}", tag="ssum")
                nc.vector.tensor_reduce(out=ssum[0:1, 0:1],
                                        in_=erow[side][0:1, b0:b0 + rows],
                                        axis=Ax.X, op=Alu.add)
                rcp = spool.tile([1, 1], f32, name=f"rc_{side}_{b}", tag="rcp")
                nc.vector.reciprocal(rcp[0:1, 0:1], ssum[0:1, 0:1])
                rb = spool.tile([128, 1], f32, name=f"rb_{side}_{b}", tag="rb")
                nc.gpsimd.partition_broadcast(rb[:, 0:1], rcp[0:1, 0:1])
                attp = attps[(side, b)]
                attf = apool.tile([128, HC], f32, name=f"attf_{side}_{b}",
                                  tag="attf")
                nc.vector.tensor_reduce(
                    out=attf[:, :],
                    in_=attp[:, :].rearrange("p (h t) -> p h t", t=NPART),
                    axis=Ax.X, op=Alu.add)
                attsc = apool.tile([128, HC], f32, name=f"attsc_{side}_{b}",
                                   tag="attsc")
                nc.scalar.activation(attsc[:, :], attf[:, :], Act.Copy,
                                     scale=rb[:, 0:1])
                atts[(side, b)] = attsc

            def emit_attT(side, b):
                """PE transpose of the output column-tile + writeback."""
                att = atts[(side, b)]
                atp = tpool.tile([8, 128], f32, name=f"atp_{side}_{b}", tag="tp")
                nc.tensor.transpose(atp[0:8, 0:128], att[:, 0:HC],
                                    ident[:, :])
                osb = spool.tile([8, 128], f32, name=f"osb_{side}_{b}", tag="osb")
                nc.scalar.activation(osb[:, :], atp[:, :], Act.Copy)
                nc.scalar.dma_start(out=out_d[SD[side]["oidx"], b],
                                    in_=osb[:, :])

            def after_T(side, ci):
                for b in range(BLOC):
                    for pi, (lo, hi, ready) in enumerate(part_plan[(side, b)]):
                        if ready == ci:
                            emit_part(side, b, lo, hi, pi)
                    if SD[side]["last"][b] == ci:
                        emit_finalize(side, b)

            # ---- emission schedule ----
            # PE order: img prologue (c0-2, hc-major, DMA-paced), img c3-c5,
            # all dns chunks (their xt arrives while img computes), and the
            # 16-row img c6 last so the end-of-kernel chain is short.  Score
            # transposes are deferred >=1 chunk; output transposes >=2.
            emit_mm_prologue("img", [0, 1, 2])
            emit_mm("img", 3)
            for ci in (0, 1, 2):
                emit_T("img", ci); after_T("img", ci)
            emit_mm("img", 4); emit_T("img", 3); after_T("img", 3)
            emit_mm("img", 5); emit_T("img", 4); after_T("img", 4)
            emit_mm("dns", 0); emit_T("img", 5); after_T("img", 5)
            emit_mm("dns", 1); emit_attT("img", 0)
            emit_mm("dns", 2); emit_attT("img", 1); emit_T("dns", 0); after_T("dns", 0)
            emit_mm("dns", 3); emit_attT("img", 2); emit_T("dns", 1); after_T("dns", 1)
            attT_slot = {7: ("dns", 0), 11: ("dns", 1), 15: ("dns", 2)}
            for ci in range(4, 15):
                emit_mm("dns", ci)
                if ci in attT_slot:
                    emit_attT(*attT_slot[ci])
                emit_T("dns", ci - 2); after_T("dns", ci - 2)
            emit_mm("dns", 15, split_chain=True)
            emit_T("dns", 13); after_T("dns", 13)
            emit_T("dns", 14); after_T("dns", 14)
            emit_mm("img", 6, split_chain=True)
            emit_attT(*attT_slot[15])
            emit_T("dns", 15); after_T("dns", 15)
            emit_T("img", 6); after_T("img", 6)
            emit_attT("dns", 3)
            emit_attT("img", 3)

    nc.compile()
    return nc


def _get_nc():
    if "nc" not in _CACHE:
        _CACHE["nc"] = build_nc()
    return _CACHE["nc"]


def make_in_maps(inputs):
    dns = np.asarray(inputs["dns_feature"], dtype=np.float32)
    img = np.asarray(inputs["img_features"], dtype=np.float32)
    W_i1 = np.asarray(inputs["W_i1"], dtype=np.float32)
    W_d2 = np.asarray(inputs["W_d2"], dtype=np.float32)
    wB = np.asarray(inputs["w_att1"], dtype=np.float32)[H:]
    wD = np.asarray(inputs["w_att2"], dtype=np.float32)[H:]

    wt_i1 = np.ascontiguousarray(W_i1.T).reshape(HC, 128, H).astype(_BF16)
    wt_d2 = np.ascontiguousarray(W_d2.T).reshape(HC, 128, H).astype(_BF16)
    wr_b = np.ascontiguousarray(np.broadcast_to(wB, (128, H)).astype(_BF16))
    wr_d = np.ascontiguousarray(np.broadcast_to(wD, (128, H)).astype(_BF16))
    ident = np.eye(128, dtype=np.float32)

    in_maps = []
    for k in range(NCORES):
        sl = slice(k * BLOC, (k + 1) * BLOC)
        xd = np.ascontiguousarray(
            dns[sl].transpose(2, 0, 1).reshape(HC, 128, DTOT).astype(_BF16))
        xi = np.ascontiguousarray(
            img[sl].transpose(2, 0, 1).reshape(HC, 128, ITOT).astype(_BF16))
        in_maps.append({
            "xt_dns": xd, "xt_img": xi,
            "wt_i1": wt_i1, "wt_d2": wt_d2,
            "wrow_b": wr_b, "wrow_d": wr_d, "ident": ident,
        })
    return in_maps


def kernel(**inputs):
    from concourse.bass_utils import run_bass_kernel_spmd

    nc = _get_nc()
    in_maps = make_in_maps(inputs)
    res = run_bass_kernel_spmd(nc, in_maps, list(range(NCORES))).results
    out = np.stack([np.asarray(res[k]["out_all"]) for k in range(NCORES)])
    img_rows = out[:, 0].reshape(B, H)
    dns_rows = out[:, 1].reshape(B, H)
    att_dns = np.ascontiguousarray(
        np.broadcast_to(dns_rows[:, None, :], (B, S, H)))
    att_img = np.ascontiguousarray(
        np.broadcast_to(img_rows[:, None, :], (B, S, H)))
    return att_dns, att_img


# revision 28
# speedup vs baseline: 1.3580x; 1.0367x over previous
"""CoAttention ImageDNS kernel for Trainium2 (8 NeuronCores, Bass/Tile).

Math: the reference computes two additive-attention blocks. In both, the
softmax'd score is  score[b, q, k] = f(q-side)[b, q] + g(k-side)[b, k] + c,
and softmax over k is invariant to the q-dependent (and constant) terms, so
the attention weights are independent of the query index:

  visual_att[b, s, :]  = softmax_r( wB . tanh(W_i1 @ img[b, r]) )
  textual_att[b, i, :] = softmax_j( wD . tanh(W_d2 @ dns[b, j]) )

Hence both outputs are per-batch rank-1 broadcasts:

  att_img_features[b, s, :] = visual_att[b]  @ img[b]   (same for all s)
  att_dns_features[b, i, :] = textual_att[b] @ dns[b]   (same for all i)

W_d1/b_d1/w_att1[:H]/b_att1/W_i2/b_i2/w_att2[:H]/b_att2 cancel entirely.

Sharding: pure data-parallel over batch, 4 batches per core, no collectives.

Device dataflow (per core), designed around the bf16 PE streaming roofline
(~216 ns per K=128 N=512 matmul; LDWEIGHTS hides under the stream):
  - Only the h-transposed activations xt[h, row] are loaded (bf16); the rows
    of all 4 batches are packed along the free dim so row-chunks of 128 have
    no per-batch padding waste (784 img rows -> 7 chunks, 2048 dns -> 16).
  - Projection: chunk-major MMs, activations stationary, weights streaming.
  - score chain per chunk: tanh (ScalarE, bf16 out) -> scalar_tensor_tensor
    with the wB/wD broadcast row + free-dim accumulate (VectorE) giving the
    score column [rk, 1]; a PE transpose turns it into a score row; exp on
    ScalarE writes the per-side exp row [1, rows].
  - per batch: row-sum + reciprocal + normalize (VectorE), partition-
    broadcast of the normalized attention row (GpSimd), then stage-2 as 8
    STT free-dim-accumulate ops over xt (VectorE) - no xn loads, no PE.
  - outputs: one [H] vector per (batch, side), PE-transposed to [8, 128]
    and DMA'd out (32 KB total instead of 16.8 MB of broadcast rows); the
    host broadcasts to the full (B, S, H) shape during unshard.
  - PE-queue ops that depend on the VectorE chain (the transposes) are
    emitted 1-2 chunks late so the in-order PE queue never stalls.
"""

import sys
import numpy as np
import ml_dtypes

_BF16 = ml_dtypes.bfloat16

for _p in ("/opt/trn_rl_repo", "/root/.axon_site/_ro/trn_rl_repo"):
    if _p not in sys.path:
        sys.path.append(_p)

B, S, R, H = 32, 512, 196, 1024
NCORES = 8
BLOC = B // NCORES          # batches per core
OC = 512                    # output-chunk (one fp32 PSUM bank)
HC = H // 128               # contraction chunks
DTOT = BLOC * S             # packed dns rows per core (2048)
ITOT = BLOC * R             # packed img rows per core (784)

_CACHE = {}


def _row_chunks(n):
    out, o = [], 0
    while o < n:
        out.append((o, min(128, n - o)))
        o += 128
    return out


def build_nc():
    from concourse import bacc, mybir
    from concourse import tile

    f32, f16 = mybir.dt.float32, mybir.dt.bfloat16
    Act = mybir.ActivationFunctionType
    Alu = mybir.AluOpType
    Ax = mybir.AxisListType

    nc = bacc.Bacc("TRN2", target_bir_lowering=False, debug=False)

    xt_dns_d = nc.dram_tensor("xt_dns", [HC, 128, DTOT], f16, kind="ExternalInput")
    xt_img_d = nc.dram_tensor("xt_img", [HC, 128, ITOT], f16, kind="ExternalInput")
    wt_i1_d = nc.dram_tensor("wt_i1", [HC, 128, H], f16, kind="ExternalInput")
    wt_d2_d = nc.dram_tensor("wt_d2", [HC, 128, H], f16, kind="ExternalInput")
    wr_b_d = nc.dram_tensor("wrow_b", [128, H], f16, kind="ExternalInput")
    wr_d_d = nc.dram_tensor("wrow_d", [128, H], f16, kind="ExternalInput")
    ident_d = nc.dram_tensor("ident", [128, 128], f32, kind="ExternalInput")
    out_d = nc.dram_tensor("out_all", [2, BLOC, HC, 128], f32, kind="ExternalOutput")
    sums_d = nc.dram_tensor("out_sums", [1, 2 * BLOC], f32, kind="ExternalOutput")

    IMG_RCS = _row_chunks(ITOT)      # 7 chunks (6x128 + 16)
    DNS_RCS = _row_chunks(DTOT)      # 16 chunks
    # batch -> last chunk holding its rows (chunk boundaries don't align with
    # batch boundaries on the img side)
    img_last_chunk = [((b + 1) * R - 1) // 128 for b in range(BLOC)]
    dns_last_chunk = [((b + 1) * S - 1) // 128 for b in range(BLOC)]

    with tile.TileContext(nc) as tc:
        with (
            tc.tile_pool(name="const", bufs=1) as cpool,
            tc.tile_pool(name="th", bufs=3) as thpool,
            tc.tile_pool(name="scr", bufs=2) as jpool,
            tc.tile_pool(name="small", bufs=3) as spool,
            tc.tile_pool(name="attp", bufs=4) as apool,
            tc.tile_pool(name="attacc", bufs=8) as accpool,
            tc.tile_pool(name="bc", bufs=2) as bpool,
            tc.tile_pool(name="pp", bufs=3, space="PSUM") as ppool,
            tc.tile_pool(name="tp", bufs=2, space="PSUM") as tpool,
        ):
            # ---- persistent SBUF tiles ----
            xt_img = cpool.tile([128, HC, ITOT], f16, name="xt_img_sb")
            xt_dns = cpool.tile([128, HC, DTOT], f16, name="xt_dns_sb")
            wt_img = cpool.tile([128, HC, H], f16, name="wt_img_sb")
            wt_dns = cpool.tile([128, HC, H], f16, name="wt_dns_sb")
            wrb = {"img": cpool.tile([128, H], f16, name="wrb_img"),
                   "dns": cpool.tile([128, H], f16, name="wrb_dns")}
            ident = cpool.tile([128, 128], f32, name="ident_sb")
            # exp rows in bf16: keeps every tensor operand of the stage-2
            # STTs 16-bit (2x DVE mode) at ~0.4% relative cost on the weights
            erow = {"img": cpool.tile([1, ITOT], f16, name="erow_img"),
                    "dns": cpool.tile([1, DTOT], f16, name="erow_dns")}
            # per-chunk exp sums (free via the exp's accum_out); dns chunks
            # align with batches so the softmax sum is a [1,4] reduce instead
            # of a [1,2048] one (2.2us at DVE 1x) on the critical tail
            esums = cpool.tile([1, len(DNS_RCS)], f32, name="esums_dns")
            # unnormalized row sums per (side, batch); normalization happens
            # on the host during the unshard step
            ssum_all = cpool.tile([1, 2 * BLOC], f32, name="ssum_all")

            SD = {
                "img": dict(xt=xt_img, wt=wt_img, rcs=IMG_RCS, rows=R,
                            last=img_last_chunk, oidx=0),
                "dns": dict(xt=xt_dns, wt=wt_dns, rcs=DNS_RCS, rows=S,
                            last=dns_last_chunk, oidx=1),
            }

            # ---- PE warmup: ~10 dummy matmuls on scratch data get the HAM
            # clock gate to 8/8 (2.4 GHz) during the first ~4us, which is
            # DMA-bound anyway; real matmuls then run warm from the start.
            warm_sb = cpool.tile([128, OC], f16, name="warm_sb")
            nc.vector.memset(warm_sb[:, :], 0.0)
            warm_ps = ppool.tile([128, H], f32, name="warm_ps", tag="pp")
            for i in range(10):
                nc.tensor.matmul(warm_ps[:, 0:OC], lhsT=warm_sb[:, 0:128],
                                 rhs=warm_sb[:, :], start=True, stop=True)

            # ---- input DMAs ----
            # three issue queues in parallel from t=0:
            #   sync:   img xt per-hc, then dns xt in two column halves
            #   gpsimd: img wt first (nothing queued ahead), then dns wt
            #   scalar: ident + wB/wD rows (small; needed only at ~10us)
            nc.scalar.dma_start(out=ident[:, :], in_=ident_d[:, :])
            nc.scalar.dma_start(out=wrb["img"][:, :], in_=wr_b_d[:, :])
            nc.scalar.dma_start(out=wrb["dns"][:, :], in_=wr_d_d[:, :])
            for hc in range(HC):
                nc.gpsimd.dma_start(out=wt_img[:, hc, :], in_=wt_i1_d[hc])
                nc.sync.dma_start(out=xt_img[:, hc, :], in_=xt_img_d[hc])
            for hc in range(HC):
                nc.gpsimd.dma_start(out=wt_dns[:, hc, :], in_=wt_d2_d[hc])
            HD = DTOT // 2
            for half in range(2):
                cs = slice(half * HD, (half + 1) * HD)
                for hc in range(HC):
                    nc.sync.dma_start(out=xt_dns[:, hc, cs],
                                      in_=xt_dns_d[hc][:, cs])

            # ---- per-chunk pieces ----
            tcols = {}
            tps_tiles = {}

            def emit_mm(side, ci, split_chain=False):
                """proj MMs for one chunk + its (non-PE) score chain.
                split_chain: oc-major MM order + per-half tanh/STT so most of
                the score chain overlaps the second half's matmuls — used for
                the final chunk of each side to shorten the kernel tail."""
                sd = SD[side]
                r0, rk = sd["rcs"][ci]
                ps = ppool.tile([128, H], f32, name=f"ps_{side}_{ci}", tag="pp")
                if not split_chain:
                    for hc in range(HC):
                        lhs = sd["xt"][:, hc, r0:r0 + rk]
                        for oc in range(2):
                            nc.tensor.matmul(
                                ps[0:rk, oc * OC:(oc + 1) * OC],
                                lhsT=lhs,
                                rhs=sd["wt"][:, hc, oc * OC:(oc + 1) * OC],
                                start=(hc == 0), stop=(hc == HC - 1))
                    emit_chain(side, ci, ps)
                    return
                halves = []
                for oc in range(2):
                    for hc in range(HC):
                        nc.tensor.matmul(
                            ps[0:rk, oc * OC:(oc + 1) * OC],
                            lhsT=sd["xt"][:, hc, r0:r0 + rk],
                            rhs=sd["wt"][:, hc, oc * OC:(oc + 1) * OC],
                            start=(hc == 0), stop=(hc == HC - 1))
                    th = thpool.tile([128, OC], f16,
                                     name=f"th_{side}_{ci}_{oc}", tag="thh")
                    nc.scalar.activation(th[0:rk, :],
                                         ps[0:rk, oc * OC:(oc + 1) * OC],
                                         Act.Tanh)
                    scr = jpool.tile([128, OC], f16,
                                     name=f"scr_{side}_{ci}_{oc}", tag="scrh")
                    tch = spool.tile([128, 1], f32,
                                     name=f"tch_{side}_{ci}_{oc}", tag="tcolh")
                    nc.vector.scalar_tensor_tensor(
                        out=scr[0:rk, :], in0=th[0:rk, :], scalar=1.0,
                        in1=wrb[side][0:rk, oc * OC:(oc + 1) * OC],
                        op0=Alu.mult, op1=Alu.mult, accum_out=tch[0:rk, :])
                    halves.append(tch)
                tcol = spool.tile([128, 1], f32, name=f"tc_{side}_{ci}",
                                  tag="tcol")
                nc.vector.scalar_tensor_tensor(
                    out=tcol[0:rk, :], in0=halves[0][0:rk, :], scalar=1.0,
                    in1=halves[1][0:rk, :], op0=Alu.mult, op1=Alu.add)
                tcols[(side, ci)] = tcol

            def emit_mm_prologue(side, cis):
                """hc-major MMs over several chunks: consumes the per-hc input
                DMAs progressively so the PE starts ~1.5us into the kernel."""
                sd = SD[side]
                pss = {}
                for ci in cis:
                    pss[ci] = ppool.tile([128, H], f32, name=f"ps_{side}_{ci}",
                                         tag="pp")
                for hc in range(HC):
                    for ci in cis:
                        r0, rk = sd["rcs"][ci]
                        lhs = sd["xt"][:, hc, r0:r0 + rk]
                        for oc in range(2):
                            nc.tensor.matmul(
                                pss[ci][0:rk, oc * OC:(oc + 1) * OC],
                                lhsT=lhs,
                                rhs=sd["wt"][:, hc, oc * OC:(oc + 1) * OC],
                                start=(hc == 0), stop=(hc == HC - 1))
                for ci in cis:
                    emit_chain(side, ci, pss[ci])

            def emit_chain(side, ci, ps):
                """tanh -> weighted free-dim reduce -> score column [rk, 1]."""
                sd = SD[side]
                r0, rk = sd["rcs"][ci]
                th = thpool.tile([128, H], f16, name=f"th_{side}_{ci}", tag="th")
                nc.scalar.activation(th[0:rk, :], ps[0:rk, :], Act.Tanh)
                scr = jpool.tile([128, H], f16, name=f"scr_{side}_{ci}", tag="scr")
                tcol = spool.tile([128, 1], f32, name=f"tc_{side}_{ci}", tag="tcol")
                nc.vector.scalar_tensor_tensor(
                    out=scr[0:rk, :], in0=th[0:rk, :], scalar=1.0,
                    in1=wrb[side][0:rk, :], op0=Alu.mult, op1=Alu.mult,
                    accum_out=tcol[0:rk, :])
                tcols[(side, ci)] = tcol

            def emit_T(side, ci):
                """PE transpose of the score column -> exp row slice.
                Emitted >=1 chunk after emit_mm so the PE queue never waits
                on the VectorE chain."""
                sd = SD[side]
                r0, rk = sd["rcs"][ci]
                tcol = tcols[(side, ci)]
                tps = tpool.tile([8, 128], f32, name=f"tps_{side}_{ci}", tag="tp")
                nc.tensor.transpose(tps[0:1, 0:rk], tcol[0:rk, 0:1],
                                    ident[0:rk, 0:rk])
                acc = esums[0:1, ci:ci + 1] if side == "dns" else None
                nc.scalar.activation(erow[side][0:1, r0:r0 + rk],
                                     tps[0:1, 0:rk], Act.Exp, accum_out=acc)

            # ---- stage 2, split into head/tail parts ----
            # att[h] = (sum_r exp_r x[h,r]) / sum_r exp_r.  The unnormalized
            # partials only need the exp row, so the head part (all chunks of
            # the batch but the last) runs a chunk earlier than a normalized
            # formulation would allow; only the last chunk's sliver plus the
            # finalize remains on the critical tail.
            attps, atts = {}, {}
            NPART = 4

            # part plan: (side, b) -> list of (lo, hi, ready_chunk).  Each
            # part becomes one broadcast + 8 STT accumulates, emitted at
            # after_T(side, ready_chunk).  Bounding every part under ~2.6us
            # of DVE time keeps the in-order DVE queue from ever delaying
            # the next chunk's score chain (one PE chunk = ~3.5us).
            part_plan = {}
            for b in range(BLOC):
                b0, bend = b * R, (b + 1) * R
                lc = img_last_chunk[b]
                if b == BLOC - 1:
                    part_plan[("img", b)] = [(b0, lc * 128, lc - 1),
                                             (lc * 128, bend, lc)]
                else:
                    part_plan[("img", b)] = [(b0, bend, lc)]
            for b in range(BLOC):
                b0, bend = b * S, (b + 1) * S
                c0 = 4 * b
                if b == BLOC - 1:
                    part_plan[("dns", b)] = [(b0, b0 + 256, c0 + 1),
                                             (b0 + 256, b0 + 384, c0 + 2),
                                             (b0 + 384, bend, c0 + 3)]
                else:
                    part_plan[("dns", b)] = [(b0, b0 + 256, c0 + 1),
                                             (b0 + 256, bend, c0 + 3)]

            def emit_part(side, b, lo, hi, pi):
                sd = SD[side]
                w = hi - lo
                key = (side, b)
                if key not in attps:
                    attp = accpool.tile([128, HC * NPART], f32,
                                        name=f"attp_{side}_{b}", tag="attp")
                    nc.vector.memset(attp[:, :], 0.0)
                    attps[key] = attp
                attp = attps[key]
                abc = bpool.tile([128, w], f16, name=f"abc_{side}_{b}_{pi}",
                                 tag=f"abc_{side}_{pi}")
                nc.gpsimd.partition_broadcast(abc[:, :], erow[side][0:1, lo:hi])
                for hc in range(HC):
                    sj = jpool.tile([128, w], f16, name=f"sj_{side}_{b}_{hc}_{pi}",
                                    tag=f"sj_{side}")
                    nc.vector.scalar_tensor_tensor(
                        out=sj[:, :], in0=sd["xt"][:, hc, lo:hi],
                        scalar=1.0, in1=abc[:, :], op0=Alu.mult, op1=Alu.mult,
                        accum_out=attp[:, hc * NPART + pi:hc * NPART + pi + 1])

            def emit_finalize(side, b):
                sd = SD[side]
                rows = sd["rows"]
                b0 = b * rows
                sidx = sd["oidx"] * BLOC + b
                if side == "dns":
                    nc.vector.tensor_reduce(
                        out=ssum_all[0:1, sidx:sidx + 1],
                        in_=esums[0:1, 4 * b:4 * b + 4], axis=Ax.X, op=Alu.add)
                else:
                    nc.vector.tensor_reduce(
                        out=ssum_all[0:1, sidx:sidx + 1],
                        in_=erow[side][0:1, b0:b0 + rows],
                        axis=Ax.X, op=Alu.add)
                attp = attps[(side, b)]
                attf = apool.tile([128, HC], f32, name=f"attf_{side}_{b}",
                                  tag="attf")
                nc.vector.tensor_reduce(
                    out=attf[:, :],
                    in_=attp[:, :].rearrange("p (h t) -> p h t", t=NPART),
                    axis=Ax.X, op=Alu.add)
                atts[(side, b)] = attf

            def emit_attT(side, b):
                """PE transpose of the output column-tile + writeback."""
                att = atts[(side, b)]
                atp = tpool.tile([8, 128], f32, name=f"atp_{side}_{b}", tag="tp")
                nc.tensor.transpose(atp[0:8, 0:128], att[:, 0:HC],
                                    ident[:, :])
                osb = spool.tile([8, 128], f32, name=f"osb_{side}_{b}", tag="osb")
                nc.scalar.activation(osb[:, :], atp[:, :], Act.Copy)
                nc.scalar.dma_start(out=out_d[SD[side]["oidx"], b],
                                    in_=osb[:, :])

            def after_T(side, ci):
                for b in range(BLOC):
                    for pi, (lo, hi, ready) in enumerate(part_plan[(side, b)]):
                        if ready == ci:
                            emit_part(side, b, lo, hi, pi)
                    if SD[side]["last"][b] == ci:
                        emit_finalize(side, b)

            # ---- emission schedule ----
            # PE order: img prologue (c0-2, hc-major, DMA-paced), img c3-c5,
            # all dns chunks (their xt arrives while img computes), and the
            # 16-row img c6 last so the end-of-kernel chain is short.  Score
            # transposes are deferred >=1 chunk; output transposes >=2.
            emit_mm_prologue("img", [0, 1, 2])
            emit_mm("img", 3)
            for ci in (0, 1, 2):
                emit_T("img", ci); after_T("img", ci)
            emit_mm("img", 4); emit_T("img", 3); after_T("img", 3)
            emit_mm("img", 5); emit_T("img", 4); after_T("img", 4)
            emit_mm("dns", 0); emit_T("img", 5); after_T("img", 5)
            emit_mm("dns", 1); emit_attT("img", 0)
            emit_mm("dns", 2); emit_attT("img", 1); emit_T("dns", 0); after_T("dns", 0)
            emit_mm("dns", 3); emit_attT("img", 2); emit_T("dns", 1); after_T("dns", 1)
            attT_slot = {7: ("dns", 0), 11: ("dns", 1), 15: ("dns", 2)}
            for ci in range(4, 15):
                emit_mm("dns", ci)
                if ci in attT_slot:
                    emit_attT(*attT_slot[ci])
                emit_T("dns", ci - 2); after_T("dns", ci - 2)
            emit_mm("dns", 15, split_chain=True)
            emit_T("dns", 13); after_T("dns", 13)
            emit_T("dns", 14); after_T("dns", 14)
            emit_mm("img", 6, split_chain=True)
            emit_attT(*attT_slot[15])
            emit_T("dns", 15); after_T("dns", 15)
            emit_T("img", 6); after_T("img", 6)
            emit_attT("dns", 3)
            emit_attT("img", 3)
            nc.scalar.dma_start(out=sums_d[:, :], in_=ssum_all[0:1, :])

    nc.compile()
    return nc


def _get_nc():
    if "nc" not in _CACHE:
        _CACHE["nc"] = build_nc()
    return _CACHE["nc"]


def make_in_maps(inputs):
    dns = np.asarray(inputs["dns_feature"], dtype=np.float32)
    img = np.asarray(inputs["img_features"], dtype=np.float32)
    W_i1 = np.asarray(inputs["W_i1"], dtype=np.float32)
    W_d2 = np.asarray(inputs["W_d2"], dtype=np.float32)
    wB = np.asarray(inputs["w_att1"], dtype=np.float32)[H:]
    wD = np.asarray(inputs["w_att2"], dtype=np.float32)[H:]

    wt_i1 = np.ascontiguousarray(W_i1.T).reshape(HC, 128, H).astype(_BF16)
    wt_d2 = np.ascontiguousarray(W_d2.T).reshape(HC, 128, H).astype(_BF16)
    wr_b = np.ascontiguousarray(np.broadcast_to(wB, (128, H)).astype(_BF16))
    wr_d = np.ascontiguousarray(np.broadcast_to(wD, (128, H)).astype(_BF16))
    ident = np.eye(128, dtype=np.float32)

    in_maps = []
    for k in range(NCORES):
        sl = slice(k * BLOC, (k + 1) * BLOC)
        xd = np.ascontiguousarray(
            dns[sl].transpose(2, 0, 1).reshape(HC, 128, DTOT).astype(_BF16))
        xi = np.ascontiguousarray(
            img[sl].transpose(2, 0, 1).reshape(HC, 128, ITOT).astype(_BF16))
        in_maps.append({
            "xt_dns": xd, "xt_img": xi,
            "wt_i1": wt_i1, "wt_d2": wt_d2,
            "wrow_b": wr_b, "wrow_d": wr_d, "ident": ident,
        })
    return in_maps


def kernel(**inputs):
    from concourse.bass_utils import run_bass_kernel_spmd

    nc = _get_nc()
    in_maps = make_in_maps(inputs)
    res = run_bass_kernel_spmd(nc, in_maps, list(range(NCORES))).results
    out = np.stack([np.asarray(res[k]["out_all"]) for k in range(NCORES)])
    sums = np.stack([np.asarray(res[k]["out_sums"]) for k in range(NCORES)])
    sums = sums.reshape(NCORES, 2, BLOC)
    img_rows = out[:, 0].reshape(B, H) / sums[:, 0].reshape(B, 1)
    dns_rows = out[:, 1].reshape(B, H) / sums[:, 1].reshape(B, 1)
    att_dns = np.ascontiguousarray(
        np.broadcast_to(dns_rows[:, None, :], (B, S, H)))
    att_img = np.ascontiguousarray(
        np.broadcast_to(img_rows[:, None, :], (B, S, H)))
    return att_dns, att_img


# revision 29
# speedup vs baseline: 1.3597x; 1.0012x over previous
"""CoAttention ImageDNS kernel for Trainium2 (8 NeuronCores, Bass/Tile).

Math: the reference computes two additive-attention blocks. In both, the
softmax'd score is  score[b, q, k] = f(q-side)[b, q] + g(k-side)[b, k] + c,
and softmax over k is invariant to the q-dependent (and constant) terms, so
the attention weights are independent of the query index:

  visual_att[b, s, :]  = softmax_r( wB . tanh(W_i1 @ img[b, r]) )
  textual_att[b, i, :] = softmax_j( wD . tanh(W_d2 @ dns[b, j]) )

Hence both outputs are per-batch rank-1 broadcasts:

  att_img_features[b, s, :] = visual_att[b]  @ img[b]   (same for all s)
  att_dns_features[b, i, :] = textual_att[b] @ dns[b]   (same for all i)

W_d1/b_d1/w_att1[:H]/b_att1/W_i2/b_i2/w_att2[:H]/b_att2 cancel entirely.

Sharding: pure data-parallel over batch, 4 batches per core, no collectives.

Device dataflow (per core), designed around the bf16 PE streaming roofline
(~216 ns per K=128 N=512 matmul; LDWEIGHTS hides under the stream):
  - Only the h-transposed activations xt[h, row] are loaded (bf16); the rows
    of all 4 batches are packed along the free dim so row-chunks of 128 have
    no per-batch padding waste (784 img rows -> 7 chunks, 2048 dns -> 16).
  - Projection: chunk-major MMs, activations stationary, weights streaming.
  - score chain per chunk: tanh (ScalarE, bf16 out) -> scalar_tensor_tensor
    with the wB/wD broadcast row + free-dim accumulate (VectorE) giving the
    score column [rk, 1]; a PE transpose turns it into a score row; exp on
    ScalarE writes the per-side exp row [1, rows].
  - per batch: row-sum + reciprocal + normalize (VectorE), partition-
    broadcast of the normalized attention row (GpSimd), then stage-2 as 8
    STT free-dim-accumulate ops over xt (VectorE) - no xn loads, no PE.
  - outputs: one [H] vector per (batch, side), PE-transposed to [8, 128]
    and DMA'd out (32 KB total instead of 16.8 MB of broadcast rows); the
    host broadcasts to the full (B, S, H) shape during unshard.
  - PE-queue ops that depend on the VectorE chain (the transposes) are
    emitted 1-2 chunks late so the in-order PE queue never stalls.
"""

import sys
import numpy as np
import ml_dtypes

_BF16 = ml_dtypes.bfloat16

for _p in ("/opt/trn_rl_repo", "/root/.axon_site/_ro/trn_rl_repo"):
    if _p not in sys.path:
        sys.path.append(_p)

B, S, R, H = 32, 512, 196, 1024
NCORES = 8
BLOC = B // NCORES          # batches per core
OC = 512                    # output-chunk (one fp32 PSUM bank)
HC = H // 128               # contraction chunks
DTOT = BLOC * S             # packed dns rows per core (2048)
ITOT = BLOC * R             # packed img rows per core (784)

_CACHE = {}


def _row_chunks(n):
    out, o = [], 0
    while o < n:
        out.append((o, min(128, n - o)))
        o += 128
    return out


def build_nc():
    from concourse import bacc, mybir
    from concourse import tile

    f32, f16 = mybir.dt.float32, mybir.dt.bfloat16
    Act = mybir.ActivationFunctionType
    Alu = mybir.AluOpType
    Ax = mybir.AxisListType

    nc = bacc.Bacc("TRN2", target_bir_lowering=False, debug=False)

    xt_dns_d = nc.dram_tensor("xt_dns", [HC, 128, DTOT], f16, kind="ExternalInput")
    xt_img_d = nc.dram_tensor("xt_img", [HC, 128, ITOT], f16, kind="ExternalInput")
    wt_i1_d = nc.dram_tensor("wt_i1", [HC, 128, H], f16, kind="ExternalInput")
    wt_d2_d = nc.dram_tensor("wt_d2", [HC, 128, H], f16, kind="ExternalInput")
    wr_b_d = nc.dram_tensor("wrow_b", [128, H], f16, kind="ExternalInput")
    wr_d_d = nc.dram_tensor("wrow_d", [128, H], f16, kind="ExternalInput")
    ident_d = nc.dram_tensor("ident", [128, 128], f32, kind="ExternalInput")
    out_d = nc.dram_tensor("out_all", [2, BLOC, HC, 128], f32, kind="ExternalOutput")
    sums_d = nc.dram_tensor("out_sums", [1, 2 * BLOC], f32, kind="ExternalOutput")

    IMG_RCS = _row_chunks(ITOT)      # 7 chunks (6x128 + 16)
    DNS_RCS = _row_chunks(DTOT)      # 16 chunks
    # batch -> last chunk holding its rows (chunk boundaries don't align with
    # batch boundaries on the img side)
    img_last_chunk = [((b + 1) * R - 1) // 128 for b in range(BLOC)]
    dns_last_chunk = [((b + 1) * S - 1) // 128 for b in range(BLOC)]

    with tile.TileContext(nc) as tc:
        with (
            tc.tile_pool(name="const", bufs=1) as cpool,
            tc.tile_pool(name="th", bufs=3) as thpool,
            tc.tile_pool(name="scr", bufs=2) as jpool,
            tc.tile_pool(name="small", bufs=3) as spool,
            tc.tile_pool(name="attp", bufs=4) as apool,
            tc.tile_pool(name="attacc", bufs=8) as accpool,
            tc.tile_pool(name="bc", bufs=2) as bpool,
            tc.tile_pool(name="pp", bufs=3, space="PSUM") as ppool,
            tc.tile_pool(name="tp", bufs=2, space="PSUM") as tpool,
        ):
            # ---- persistent SBUF tiles ----
            xt_img = cpool.tile([128, HC, ITOT], f16, name="xt_img_sb")
            xt_dns = cpool.tile([128, HC, DTOT], f16, name="xt_dns_sb")
            wt_img = cpool.tile([128, HC, H], f16, name="wt_img_sb")
            wt_dns = cpool.tile([128, HC, H], f16, name="wt_dns_sb")
            wrb = {"img": cpool.tile([128, H], f16, name="wrb_img"),
                   "dns": cpool.tile([128, H], f16, name="wrb_dns")}
            ident = cpool.tile([128, 128], f32, name="ident_sb")
            # exp rows in bf16: keeps every tensor operand of the stage-2
            # STTs 16-bit (2x DVE mode) at ~0.4% relative cost on the weights
            erow = {"img": cpool.tile([1, ITOT], f16, name="erow_img"),
                    "dns": cpool.tile([1, DTOT], f16, name="erow_dns")}
            # per-chunk exp sums (free via the exp's accum_out); dns chunks
            # align with batches so the softmax sum is a [1,4] reduce instead
            # of a [1,2048] one (2.2us at DVE 1x) on the critical tail
            esums = cpool.tile([1, len(DNS_RCS)], f32, name="esums_dns")
            # unnormalized row sums per (side, batch); normalization happens
            # on the host during the unshard step
            ssum_all = cpool.tile([1, 2 * BLOC], f32, name="ssum_all")

            SD = {
                "img": dict(xt=xt_img, wt=wt_img, rcs=IMG_RCS, rows=R,
                            last=img_last_chunk, oidx=0),
                "dns": dict(xt=xt_dns, wt=wt_dns, rcs=DNS_RCS, rows=S,
                            last=dns_last_chunk, oidx=1),
            }

            # ---- PE warmup: ~10 dummy matmuls on scratch data get the HAM
            # clock gate to 8/8 (2.4 GHz) during the first ~4us, which is
            # DMA-bound anyway; real matmuls then run warm from the start.
            warm_sb = cpool.tile([128, OC], f16, name="warm_sb")
            nc.vector.memset(warm_sb[:, :], 0.0)
            warm_ps = ppool.tile([128, H], f32, name="warm_ps", tag="pp")
            for i in range(10):
                nc.tensor.matmul(warm_ps[:, 0:OC], lhsT=warm_sb[:, 0:128],
                                 rhs=warm_sb[:, :], start=True, stop=True)

            # ---- input DMAs ----
            # three issue queues in parallel from t=0:
            #   sync:   img xt per-hc, then dns xt in two column halves
            #   gpsimd: img wt first (nothing queued ahead), then dns wt
            #   scalar: ident + wB/wD rows (small; needed only at ~10us)
            nc.scalar.dma_start(out=ident[:, :], in_=ident_d[:, :])
            nc.scalar.dma_start(out=wrb["img"][:, :], in_=wr_b_d[:, :])
            nc.scalar.dma_start(out=wrb["dns"][:, :], in_=wr_d_d[:, :])
            for hc in range(HC):
                nc.gpsimd.dma_start(out=wt_img[:, hc, :], in_=wt_i1_d[hc])
                nc.sync.dma_start(out=xt_img[:, hc, :], in_=xt_img_d[hc])
            for hc in range(HC):
                nc.gpsimd.dma_start(out=wt_dns[:, hc, :], in_=wt_d2_d[hc])
            HD = DTOT // 2
            for half in range(2):
                cs = slice(half * HD, (half + 1) * HD)
                for hc in range(HC):
                    nc.sync.dma_start(out=xt_dns[:, hc, cs],
                                      in_=xt_dns_d[hc][:, cs])

            # ---- per-chunk pieces ----
            tcols = {}
            tps_tiles = {}

            def emit_mm(side, ci, split_chain=False):
                """proj MMs for one chunk + its (non-PE) score chain.
                split_chain: oc-major MM order + per-half tanh/STT so most of
                the score chain overlaps the second half's matmuls — used for
                the final chunk of each side to shorten the kernel tail."""
                sd = SD[side]
                r0, rk = sd["rcs"][ci]
                ps = ppool.tile([128, H], f32, name=f"ps_{side}_{ci}", tag="pp")
                if not split_chain:
                    for hc in range(HC):
                        lhs = sd["xt"][:, hc, r0:r0 + rk]
                        for oc in range(2):
                            nc.tensor.matmul(
                                ps[0:rk, oc * OC:(oc + 1) * OC],
                                lhsT=lhs,
                                rhs=sd["wt"][:, hc, oc * OC:(oc + 1) * OC],
                                start=(hc == 0), stop=(hc == HC - 1))
                    emit_chain(side, ci, ps)
                    return
                halves = []
                for oc in range(2):
                    for hc in range(HC):
                        nc.tensor.matmul(
                            ps[0:rk, oc * OC:(oc + 1) * OC],
                            lhsT=sd["xt"][:, hc, r0:r0 + rk],
                            rhs=sd["wt"][:, hc, oc * OC:(oc + 1) * OC],
                            start=(hc == 0), stop=(hc == HC - 1))
                    th = thpool.tile([128, OC], f16,
                                     name=f"th_{side}_{ci}_{oc}", tag="thh")
                    nc.scalar.activation(th[0:rk, :],
                                         ps[0:rk, oc * OC:(oc + 1) * OC],
                                         Act.Tanh)
                    scr = jpool.tile([128, OC], f16,
                                     name=f"scr_{side}_{ci}_{oc}", tag="scrh")
                    tch = spool.tile([128, 1], f32,
                                     name=f"tch_{side}_{ci}_{oc}", tag="tcolh")
                    nc.vector.scalar_tensor_tensor(
                        out=scr[0:rk, :], in0=th[0:rk, :], scalar=1.0,
                        in1=wrb[side][0:rk, oc * OC:(oc + 1) * OC],
                        op0=Alu.mult, op1=Alu.mult, accum_out=tch[0:rk, :])
                    halves.append(tch)
                tcol = spool.tile([128, 1], f32, name=f"tc_{side}_{ci}",
                                  tag="tcol")
                nc.vector.scalar_tensor_tensor(
                    out=tcol[0:rk, :], in0=halves[0][0:rk, :], scalar=1.0,
                    in1=halves[1][0:rk, :], op0=Alu.mult, op1=Alu.add)
                tcols[(side, ci)] = tcol

            def emit_mm_prologue(side, cis):
                """hc-major MMs over several chunks: consumes the per-hc input
                DMAs progressively so the PE starts ~1.5us into the kernel."""
                sd = SD[side]
                pss = {}
                for ci in cis:
                    pss[ci] = ppool.tile([128, H], f32, name=f"ps_{side}_{ci}",
                                         tag="pp")
                for hc in range(HC):
                    for ci in cis:
                        r0, rk = sd["rcs"][ci]
                        lhs = sd["xt"][:, hc, r0:r0 + rk]
                        for oc in range(2):
                            nc.tensor.matmul(
                                pss[ci][0:rk, oc * OC:(oc + 1) * OC],
                                lhsT=lhs,
                                rhs=sd["wt"][:, hc, oc * OC:(oc + 1) * OC],
                                start=(hc == 0), stop=(hc == HC - 1))
                for ci in cis:
                    emit_chain(side, ci, pss[ci])

            def emit_chain(side, ci, ps):
                """tanh -> weighted free-dim reduce -> score column [rk, 1]."""
                sd = SD[side]
                r0, rk = sd["rcs"][ci]
                th = thpool.tile([128, H], f16, name=f"th_{side}_{ci}", tag="th")
                nc.scalar.activation(th[0:rk, :], ps[0:rk, :], Act.Tanh)
                scr = jpool.tile([128, H], f16, name=f"scr_{side}_{ci}", tag="scr")
                tcol = spool.tile([128, 1], f32, name=f"tc_{side}_{ci}", tag="tcol")
                nc.vector.scalar_tensor_tensor(
                    out=scr[0:rk, :], in0=th[0:rk, :], scalar=1.0,
                    in1=wrb[side][0:rk, :], op0=Alu.mult, op1=Alu.mult,
                    accum_out=tcol[0:rk, :])
                tcols[(side, ci)] = tcol

            def emit_T(side, ci):
                """PE transpose of the score column -> exp row slice.
                Emitted >=1 chunk after emit_mm so the PE queue never waits
                on the VectorE chain."""
                sd = SD[side]
                r0, rk = sd["rcs"][ci]
                tcol = tcols[(side, ci)]
                tps = tpool.tile([8, 128], f32, name=f"tps_{side}_{ci}", tag="tp")
                nc.tensor.transpose(tps[0:1, 0:rk], tcol[0:rk, 0:1],
                                    ident[0:rk, 0:rk])
                acc = esums[0:1, ci:ci + 1] if side == "dns" else None
                nc.scalar.activation(erow[side][0:1, r0:r0 + rk],
                                     tps[0:1, 0:rk], Act.Exp, accum_out=acc)

            # ---- stage 2, split into head/tail parts ----
            # att[h] = (sum_r exp_r x[h,r]) / sum_r exp_r.  The unnormalized
            # partials only need the exp row, so the head part (all chunks of
            # the batch but the last) runs a chunk earlier than a normalized
            # formulation would allow; only the last chunk's sliver plus the
            # finalize remains on the critical tail.
            attps, atts = {}, {}
            NPART = 4

            # part plan: (side, b) -> list of (lo, hi, ready_chunk).  Each
            # part becomes one broadcast + 8 STT accumulates, emitted at
            # after_T(side, ready_chunk).  Bounding every part under ~2.6us
            # of DVE time keeps the in-order DVE queue from ever delaying
            # the next chunk's score chain (one PE chunk = ~3.5us).
            part_plan = {}
            for b in range(BLOC):
                b0, bend = b * R, (b + 1) * R
                lc = img_last_chunk[b]
                if b == BLOC - 1:
                    part_plan[("img", b)] = [(b0, lc * 128, lc - 1),
                                             (lc * 128, bend, lc)]
                else:
                    part_plan[("img", b)] = [(b0, bend, lc)]
            for b in range(BLOC):
                b0, bend = b * S, (b + 1) * S
                c0 = 4 * b
                if b == BLOC - 1:
                    part_plan[("dns", b)] = [(b0, b0 + 256, c0 + 1),
                                             (b0 + 256, b0 + 384, c0 + 2),
                                             (b0 + 384, bend, c0 + 3)]
                else:
                    part_plan[("dns", b)] = [(b0, b0 + 256, c0 + 1),
                                             (b0 + 256, bend, c0 + 3)]

            def emit_part(side, b, lo, hi, pi):
                sd = SD[side]
                w = hi - lo
                key = (side, b)
                if key not in attps:
                    attp = accpool.tile([128, HC * NPART], f32,
                                        name=f"attp_{side}_{b}", tag="attp")
                    nc.vector.memset(attp[:, :], 0.0)
                    attps[key] = attp
                attp = attps[key]
                abc = bpool.tile([128, w], f16, name=f"abc_{side}_{b}_{pi}",
                                 tag=f"abc_{side}_{pi}")
                nc.gpsimd.partition_broadcast(abc[:, :], erow[side][0:1, lo:hi])
                for hc in range(HC):
                    sj = jpool.tile([128, w], f16, name=f"sj_{side}_{b}_{hc}_{pi}",
                                    tag=f"sj_{side}")
                    nc.vector.scalar_tensor_tensor(
                        out=sj[:, :], in0=sd["xt"][:, hc, lo:hi],
                        scalar=1.0, in1=abc[:, :], op0=Alu.mult, op1=Alu.mult,
                        accum_out=attp[:, hc * NPART + pi:hc * NPART + pi + 1])

            def emit_finalize(side, b):
                sd = SD[side]
                rows = sd["rows"]
                b0 = b * rows
                sidx = sd["oidx"] * BLOC + b
                if side == "dns":
                    nc.vector.tensor_reduce(
                        out=ssum_all[0:1, sidx:sidx + 1],
                        in_=esums[0:1, 4 * b:4 * b + 4], axis=Ax.X, op=Alu.add)
                else:
                    nc.vector.tensor_reduce(
                        out=ssum_all[0:1, sidx:sidx + 1],
                        in_=erow[side][0:1, b0:b0 + rows],
                        axis=Ax.X, op=Alu.add)
                attp = attps[(side, b)]
                attf = apool.tile([128, HC], f32, name=f"attf_{side}_{b}",
                                  tag="attf")
                nc.vector.tensor_reduce(
                    out=attf[:, :],
                    in_=attp[:, :].rearrange("p (h t) -> p h t", t=NPART),
                    axis=Ax.X, op=Alu.add)
                atts[(side, b)] = attf

            def emit_attT(side, b):
                """PE transpose of the output column-tile + writeback."""
                att = atts[(side, b)]
                atp = tpool.tile([8, 128], f32, name=f"atp_{side}_{b}", tag="tp")
                nc.tensor.transpose(atp[0:8, 0:128], att[:, 0:HC],
                                    ident[:, :])
                osb = spool.tile([8, 128], f32, name=f"osb_{side}_{b}", tag="osb")
                nc.scalar.activation(osb[:, :], atp[:, :], Act.Copy)
                nc.scalar.dma_start(out=out_d[SD[side]["oidx"], b],
                                    in_=osb[:, :])

            def after_T(side, ci):
                for b in range(BLOC):
                    for pi, (lo, hi, ready) in enumerate(part_plan[(side, b)]):
                        if ready == ci:
                            emit_part(side, b, lo, hi, pi)
                    if SD[side]["last"][b] == ci:
                        emit_finalize(side, b)

            # ---- emission schedule ----
            # PE order: img prologue (c0-2, hc-major, DMA-paced), img c3-c5,
            # all dns chunks (their xt arrives while img computes), and the
            # 16-row img c6 last so the end-of-kernel chain is short.  Score
            # transposes are deferred >=1 chunk; output transposes >=2.
            emit_mm_prologue("img", [0, 1, 2])
            emit_mm("img", 3)
            for ci in (0, 1, 2):
                emit_T("img", ci); after_T("img", ci)
            emit_mm("img", 4); emit_T("img", 3); after_T("img", 3)
            emit_mm("img", 5); emit_T("img", 4); after_T("img", 4)
            emit_mm("dns", 0); emit_T("img", 5); after_T("img", 5)
            emit_mm("dns", 1); emit_attT("img", 0)
            emit_mm("dns", 2); emit_attT("img", 1); emit_T("dns", 0); after_T("dns", 0)
            emit_mm("dns", 3); emit_attT("img", 2); emit_T("dns", 1); after_T("dns", 1)
            attT_slot = {7: ("dns", 0), 11: ("dns", 1), 15: ("dns", 2)}
            for ci in range(4, 15):
                emit_mm("dns", ci)
                if ci in attT_slot:
                    emit_attT(*attT_slot[ci])
                emit_T("dns", ci - 2); after_T("dns", ci - 2)
            # end on the dns side: img c6 runs second-to-last so the whole
            # img-side tail (part + finalize + transpose) hides under the
            # final dns chunk's matmuls
            emit_mm("img", 6, split_chain=True)
            emit_T("dns", 13); after_T("dns", 13)
            emit_T("dns", 14); after_T("dns", 14)
            emit_mm("dns", 15, split_chain=True)
            emit_attT(*attT_slot[15])
            emit_T("img", 6); after_T("img", 6)
            emit_T("dns", 15); after_T("dns", 15)
            emit_attT("img", 3)
            emit_attT("dns", 3)
            nc.scalar.dma_start(out=sums_d[:, :], in_=ssum_all[0:1, :])

    nc.compile()
    return nc


def _get_nc():
    if "nc" not in _CACHE:
        _CACHE["nc"] = build_nc()
    return _CACHE["nc"]


def make_in_maps(inputs):
    dns = np.asarray(inputs["dns_feature"], dtype=np.float32)
    img = np.asarray(inputs["img_features"], dtype=np.float32)
    W_i1 = np.asarray(inputs["W_i1"], dtype=np.float32)
    W_d2 = np.asarray(inputs["W_d2"], dtype=np.float32)
    wB = np.asarray(inputs["w_att1"], dtype=np.float32)[H:]
    wD = np.asarray(inputs["w_att2"], dtype=np.float32)[H:]

    wt_i1 = np.ascontiguousarray(W_i1.T).reshape(HC, 128, H).astype(_BF16)
    wt_d2 = np.ascontiguousarray(W_d2.T).reshape(HC, 128, H).astype(_BF16)
    wr_b = np.ascontiguousarray(np.broadcast_to(wB, (128, H)).astype(_BF16))
    wr_d = np.ascontiguousarray(np.broadcast_to(wD, (128, H)).astype(_BF16))
    ident = np.eye(128, dtype=np.float32)

    in_maps = []
    for k in range(NCORES):
        sl = slice(k * BLOC, (k + 1) * BLOC)
        xd = np.ascontiguousarray(
            dns[sl].transpose(2, 0, 1).reshape(HC, 128, DTOT).astype(_BF16))
        xi = np.ascontiguousarray(
            img[sl].transpose(2, 0, 1).reshape(HC, 128, ITOT).astype(_BF16))
        in_maps.append({
            "xt_dns": xd, "xt_img": xi,
            "wt_i1": wt_i1, "wt_d2": wt_d2,
            "wrow_b": wr_b, "wrow_d": wr_d, "ident": ident,
        })
    return in_maps


def kernel(**inputs):
    from concourse.bass_utils import run_bass_kernel_spmd

    nc = _get_nc()
    in_maps = make_in_maps(inputs)
    res = run_bass_kernel_spmd(nc, in_maps, list(range(NCORES))).results
    out = np.stack([np.asarray(res[k]["out_all"]) for k in range(NCORES)])
    sums = np.stack([np.asarray(res[k]["out_sums"]) for k in range(NCORES)])
    sums = sums.reshape(NCORES, 2, BLOC)
    img_rows = out[:, 0].reshape(B, H) / sums[:, 0].reshape(B, 1)
    dns_rows = out[:, 1].reshape(B, H) / sums[:, 1].reshape(B, 1)
    att_dns = np.ascontiguousarray(
        np.broadcast_to(dns_rows[:, None, :], (B, S, H)))
    att_img = np.ascontiguousarray(
        np.broadcast_to(img_rows[:, None, :], (B, S, H)))
    return att_dns, att_img
